# revision 25
# baseline (speedup 1.0000x reference)
"""Two-layer single-head GAT (PyG GATConv semantics) on 8 Trainium2 NeuronCores.

Strategy (dst-sharded edge-parallel):
  * Host: add self-loops, sort edges by destination, shard destinations
    across the 8 cores (12500 nodes each), and pack each core's edges into
    fixed 512-edge tiles such that every destination's incoming-edge segment
    lies entirely inside one tile and each tile uses at most 64 segment
    slots.  All structure (gather indices, local segment ids, scatter
    indices) is precomputed as index arrays; all floating-point math runs on
    device.
  * Device phase 0: h1aug = x @ [W1 | W1@att_src1 | W1@att_dst1] for the
    core's node shard -> AllGather to replicate the full [N, 66] table.
  * Edge phase (per layer): indirect-DMA gather of per-edge source rows
    [h | a_src] plus destination a_dst scalars, attention logits
    e = leakyrelu(a_s + a_d), p = exp(e) (no max subtraction needed: |e|<~4),
    one-hot segment matmul accumulates [p*h | p] per destination into PSUM,
    normalization + bias (+relu + W2 projection for layer 1), transpose, and
    indirect-DMA scatter into the core's output shard.
  * AllGather of the layer-1 output table [g2 | a_src2 | a_dst2] feeds the
    identical layer-2 edge phase, which scatters the final [12500, 40] shard.
"""

import numpy as np

N_NODES = 100000
N_CORES = 8
F_IN, H, C = 128, 64, 40

TILE_E = 512            # edges per edge-tile
KPART = TILE_E // 128   # 4 edge slots per partition
SEGCAP = 64             # destination-segment slots per edge-tile
SUPER = 8               # edge-tiles per super-tile (SUPER*SEGCAP = 512 psum cols)
DUMMY_SEG = 99.0        # segid for padding edges: matches no one-hot column
SENT = 1 << 24          # scatter index sentinel (> bounds_check -> skipped)

DEF_CFG = dict(
    n=N_NODES, nshard=N_NODES // N_CORES, fin=F_IN, h=H, c=C,
    tile_e=TILE_E, k=KPART, segcap=SEGCAP, sup=SUPER, ntw=500,
)


# ----------------------------------------------------------------- host prep
def _pack_core(src_c, dst_c, base, cfg):
    nshard, tile_e, segcap = cfg["nshard"], cfg["tile_e"], cfg["segcap"]
    counts = np.bincount(dst_c - base, minlength=nshard)
    assert counts.min() >= 1 and counts.max() <= tile_e
    cum = np.concatenate([[0], np.cumsum(counts)])
    tiles = []
    i = 0
    while i < nshard:
        j = int(np.searchsorted(cum, cum[i] + tile_e, side="right")) - 1
        j = min(j, i + segcap, nshard)
        assert j > i
        tiles.append((i, j))
        i = j
    T = len(tiles)
    src_g = np.zeros((T, tile_e), np.int32)
    dst_g = np.zeros((T, tile_e), np.int32)
    segid = np.full((T, tile_e), DUMMY_SEG, np.float32)
    dstseg = np.full((T, segcap), -1, np.int64)
    for t, (i, j) in enumerate(tiles):
        e0, e1 = int(cum[i]), int(cum[j])
        n = e1 - e0
        src_g[t, :n] = src_c[e0:e1]
        dst_g[t, :n] = dst_c[e0:e1]
        segid[t, :n] = (dst_c[e0:e1] - base - i).astype(np.float32)
        dstseg[t, : j - i] = np.arange(i, j) - 0 + base
    return src_g, dst_g, segid, dstseg


def _edge_layout(arr_t):  # [T, tile_e] -> [128, T*K] with (t, k*128+p) -> [p, K*t+k]
    T, tile_e = arr_t.shape
    k = tile_e // 128
    return np.ascontiguousarray(
        arr_t.reshape(T, k, 128).transpose(2, 0, 1).reshape(128, T * k)
    )


def preprocess(edge_index, cfg):
    n, nshard, sup, segcap = cfg["n"], cfg["nshard"], cfg["sup"], cfg["segcap"]
    src = np.asarray(edge_index[0]).astype(np.int64)
    dst = np.asarray(edge_index[1]).astype(np.int64)
    loop = np.arange(n, dtype=np.int64)
    src = np.concatenate([src, loop])
    dst = np.concatenate([dst, loop])
    order = np.argsort(dst, kind="stable")
    src, dst = src[order], dst[order]
    bounds = np.searchsorted(dst, np.arange(N_CORES + 1) * nshard)
    packed = [
        _pack_core(src[bounds[c]: bounds[c + 1]], dst[bounds[c]: bounds[c + 1]],
                   c * nshard, cfg)
        for c in range(N_CORES)
    ]
    Tmax = max(p[0].shape[0] for p in packed)
    Tmax = -(-Tmax // sup) * sup
    cores = []
    for c, (src_g, dst_g, segid, dstseg) in enumerate(packed):
        pad = Tmax - src_g.shape[0]
        src_g = np.pad(src_g, ((0, pad), (0, 0)))
        dst_g = np.pad(dst_g, ((0, pad), (0, 0)))
        segid = np.pad(segid, ((0, pad), (0, 0)), constant_values=DUMMY_SEG)
        dstseg = np.pad(dstseg, ((0, pad), (0, 0)), constant_values=-1)
        # scatter indices: per super-tile, 512 seg slots -> local dst or SENT
        segs = dstseg.reshape(Tmax // sup, sup * segcap)
        scat = np.where(segs >= 0, segs - c * nshard, SENT).astype(np.int32)
        scat = _edge_layout(scat)  # [128, (T//sup)*4]
        cores.append(dict(
            srcg=_edge_layout(src_g),
            dstg=_edge_layout(dst_g),
            segid=_edge_layout(segid).astype(np.float32),
            scat=scat,
        ))
    return cores, Tmax


def _compress_deps(nc):
    """Drop redundant sync dependencies so walrus' per-instruction HW wait
    slots don't overflow.  Producers on the same engine execute in issue
    order, and DMAs on the same logical queue complete in FIFO order, so a
    dependency on the latest producer of each stream subsumes the earlier
    ones.  Collectives are never dropped."""
    f = nc.m.functions[0]

    def all_insts(blk):
        for i in blk.instructions:
            yield i
        for sb in getattr(blk, "blocks", []) or []:
            yield from all_insts(sb)

    insts = [i for b in f.blocks for i in all_insts(b)]
    pos = {i.name: p for p, i in enumerate(insts)}
    by_name = {i.name: i for i in insts}

    def stream_key(p):
        tname = type(p).__name__
        if tname == "InstCollectiveCompute":
            return None  # own completion semaphore; never compress
        if tname == "InstDMACopy":
            return ("dma", str(getattr(p, "queue", "")), str(p.engine))
        return ("eng", str(p.engine))

    for i in insts:
        deps = list(i.sync_dependency_names())
        if len(deps) <= 2:
            continue
        best: dict = {}
        keep = []
        for d in deps:
            p = by_name.get(d)
            if p is None:
                keep.append(d)
                continue
            k = stream_key(p)
            if k is None:
                keep.append(d)
                continue
            cur = best.get(k)
            if cur is None or pos[d] > pos[cur]:
                best[k] = d
        keep += list(best.values())
        for d in deps:
            if d not in keep:
                i.try_remove_dependency(d)


# ------------------------------------------------------------- device program
def build_program(cfg, T, debug=False):
    import concourse.bass as bass
    import concourse.bacc as bacc
    import concourse.mybir as mybir
    import concourse.tile as tile
    from concourse.masks import make_identity

    f32 = mybir.dt.float32
    i32 = mybir.dt.int32
    nshard, fin, h, c = cfg["nshard"], cfg["fin"], cfg["h"], cfg["c"]
    k, segcap, sup, ntw = cfg["k"], cfg["segcap"], cfg["sup"], cfg["ntw"]
    n = cfg["n"]
    nsup = T // sup
    qw = ntw // 4  # phase-0 transpose quad width

    nc = bacc.Bacc(
        "TRN2", target_bir_lowering=False, debug=False,
        enable_asserts=False, num_devices=N_CORES,
    )

    xT = nc.dram_tensor("xT", [fin, nshard], f32, kind="ExternalInput").ap()
    w1aug = nc.dram_tensor("w1aug", [fin, h + 2], f32, kind="ExternalInput").ap()
    w2aug = nc.dram_tensor("w2aug", [h, c + 2], f32, kind="ExternalInput").ap()
    b1 = nc.dram_tensor("b1", [h, 1], f32, kind="ExternalInput").ap()
    b2 = nc.dram_tensor("b2", [c, 1], f32, kind="ExternalInput").ap()
    srcg = nc.dram_tensor("srcg", [128, T * k], i32, kind="ExternalInput").ap()
    dstg = nc.dram_tensor("dstg", [128, T * k], i32, kind="ExternalInput").ap()
    segid = nc.dram_tensor("segid", [128, T * k], f32, kind="ExternalInput").ap()
    scat = nc.dram_tensor("scat", [128, nsup * k], i32, kind="ExternalInput").ap()
    out2 = nc.dram_tensor("out2", [nshard, c], f32, kind="ExternalOutput").ap()

    with tile.TileContext(nc) as tc:
        with (
            tc.tile_pool(name="consts", bufs=1) as cpool,
            tc.tile_pool(name="work", bufs=2 * sup) as wpool,
            tc.tile_pool(name="epil", bufs=3) as epool,
            tc.tile_pool(name="psum", bufs=2, space="PSUM") as pp,
            tc.tile_pool(name="dram", bufs=1, space="DRAM") as dpool,
        ):
            # ---- constants / panels
            w1aug_sb = cpool.tile([fin, h + 2], f32, name="w1aug_sb")
            nc.sync.dma_start(w1aug_sb[:], w1aug)
            w2aug_sb = cpool.tile([h, c + 2], f32, name="w2aug_sb")
            nc.sync.dma_start(w2aug_sb[:], w2aug)
            b1_sb = cpool.tile([h, 1], f32, name="b1_sb")
            nc.sync.dma_start(b1_sb[:], b1)
            b2_sb = cpool.tile([c, 1], f32, name="b2_sb")
            nc.sync.dma_start(b2_sb[:], b2)
            ident = cpool.tile([128, 128], f32, name="ident")
            make_identity(nc, ident[:])
            ones_sb = cpool.tile([1, h], f32, name="ones_sb")
            nc.vector.memset(ones_sb[:], 1.0)
            iota_i = cpool.tile([128, segcap], i32, name="iota_i")
            nc.gpsimd.iota(iota_i[:], pattern=[[1, segcap]], base=0,
                           channel_multiplier=0)
            iota_f = cpool.tile([128, segcap], f32, name="iota_f")
            nc.vector.tensor_copy(iota_f[:], iota_i[:])
            srcg_sb = cpool.tile([128, T * k], i32, name="srcg_sb")
            nc.sync.dma_start(srcg_sb[:], srcg)
            dstg_sb = cpool.tile([128, T * k], i32, name="dstg_sb")
            nc.sync.dma_start(dstg_sb[:], dstg)
            segid_sb = cpool.tile([128, T * k], f32, name="segid_sb")
            nc.sync.dma_start(segid_sb[:], segid)
            scat_sb = cpool.tile([128, nsup * k], i32, name="scat_sb")
            nc.sync.dma_start(scat_sb[:], scat)

            bound_reg = nc.gpsimd.to_reg(nshard - 1)

            # ---- internal DRAM tables
            h1s = dpool.tile([nshard, h + 2], f32, name="h1s")
            h1f = dpool.tile([n, h + 2], f32, name="h1f", addr_space="Shared")
            g2s = dpool.tile([nshard, c + 2], f32, name="g2s")
            g2f = dpool.tile([n, c + 2], f32, name="g2f", addr_space="Shared")

            # ---- phase 0: h1aug shard = (x @ W1aug) rows for this shard
            for nt in range(nshard // ntw):
                o = nt * ntw
                xt = epool.tile([fin, ntw], f32, name="xt")
                nc.sync.dma_start(xt[:], xT[:, o:o + ntw])
                psH = pp.tile([h + 2, ntw], f32, name="psH", tag="pA")
                nc.tensor.matmul(psH[:], lhsT=w1aug_sb[:], rhs=xt[:],
                                 start=True, stop=True)
                h1t = epool.tile([h + 2, ntw], f32, name="h1t")
                nc.vector.tensor_copy(h1t[:], psH[:])
                psT = pp.tile([qw, 4 * (h + 2)], f32, name="psT", tag="pD")
                for q in range(4):
                    nc.tensor.transpose(
                        psT[:, q * (h + 2):(q + 1) * (h + 2)],
                        in_=h1t[:, q * qw:(q + 1) * qw],
                        identity=ident[0:h + 2, 0:h + 2],
                    )
                h1r = epool.tile([qw, 4 * (h + 2)], f32, name="h1r")
                nc.vector.tensor_copy(h1r[:], psT[:])
                for q in range(4):
                    nc.sync.dma_start(
                        h1s[o + q * qw:o + (q + 1) * qw, :],
                        h1r[:, q * (h + 2):(q + 1) * (h + 2)],
                    )

            nc.gpsimd.collective_compute(
                "AllGather", mybir.AluOpType.bypass,
                replica_groups=[list(range(N_CORES))],
                ins=[h1s[:]], outs=[h1f[:]],
            )

            # ---- edge phases
            def edge_layer(table, fdim, out_shard, last):
                """fdim: feature width of gathered rows (h or c); gathers
                fdim+1 elems [feat | a_src] per edge from table [n, fdim+2].
                The aggregation matmul is padded to mw=65 columns so the
                denominator lands at PSUM partition 64 (PSUM reads must start
                at a multiple of 32)."""
                rw = fdim + 1
                mw = 65
                for S in range(nsup):
                    ts = [S * sup + tau for tau in range(sup)]
                    # HW indirect DMA: one index per partition per call, each
                    # reading a contiguous block -> one call per 128-edge chunk
                    rows = [wpool.tile([128, k * rw], f32, name=f"rows{last}")
                            for _ in ts]
                    for i, t in enumerate(ts):
                        for kk in range(k):
                            nc.gpsimd.indirect_dma_start(
                                out=rows[i][:, kk * rw:(kk + 1) * rw],
                                out_offset=None,
                                in_=table[:],
                                in_offset=bass.IndirectOffsetOnAxis(
                                    ap=srcg_sb[:, k * t + kk:k * t + kk + 1],
                                    axis=0),
                                element_offset=0,
                            )
                    ad = wpool.tile([128, sup * k], f32, name=f"ad{last}")
                    for i, t in enumerate(ts):
                        for kk in range(k):
                            col = i * k + kk
                            nc.gpsimd.indirect_dma_start(
                                out=ad[:, col:col + 1], out_offset=None,
                                in_=table[:],
                                in_offset=bass.IndirectOffsetOnAxis(
                                    ap=dstg_sb[:, k * t + kk:k * t + kk + 1],
                                    axis=0),
                                element_offset=rw,
                            )
                    es = [wpool.tile([128, k], f32, name=f"es{last}") for _ in ts]
                    for i in range(sup):
                        rv = rows[i][:].rearrange("p (k f) -> p k f", f=rw)
                        nc.vector.tensor_tensor(
                            out=es[i][:], in0=rv[:, :, fdim],
                            in1=ad[:, i * k:(i + 1) * k],
                            op=mybir.AluOpType.add)
                    e2s = [wpool.tile([128, k], f32, name=f"e2{last}") for _ in ts]
                    for i in range(sup):
                        nc.vector.tensor_scalar_mul(e2s[i][:], es[i][:], 0.2)
                    for i in range(sup):
                        nc.vector.tensor_tensor(out=es[i][:], in0=es[i][:],
                                                in1=e2s[i][:],
                                                op=mybir.AluOpType.max)
                    ps = [wpool.tile([128, k], f32, name=f"p{last}") for _ in ts]
                    for i in range(sup):
                        nc.scalar.activation(ps[i][:], es[i][:],
                                             mybir.ActivationFunctionType.Exp)
                    for i in range(sup):
                        rv = rows[i][:].rearrange("p (k f) -> p k f", f=rw)
                        nc.vector.memset(rv[:, :, fdim], 1.0)
                    wrows = [wpool.tile([128, k * mw], f32, name=f"wr{last}")
                             for _ in ts]
                    if rw < mw:
                        for i in range(sup):
                            nc.vector.memset(wrows[i][:], 0.0)
                    for i in range(sup):
                        for kk in range(k):
                            nc.vector.tensor_scalar_mul(
                                wrows[i][:, kk * mw:kk * mw + rw],
                                rows[i][:, kk * rw:(kk + 1) * rw],
                                ps[i][:, kk:kk + 1])
                    if rw < mw:
                        for i in range(sup):
                            wv = wrows[i][:].rearrange("p (k f) -> p k f", f=mw)
                            nc.vector.tensor_copy(wv[:, :, mw - 1], ps[i][:])
                    ohs = [wpool.tile([128, k * segcap], f32, name=f"oh{last}")
                           for _ in ts]
                    for i, t in enumerate(ts):
                        for kk in range(k):
                            nc.vector.tensor_scalar(
                                ohs[i][:, kk * segcap:(kk + 1) * segcap],
                                iota_f[:],
                                segid_sb[:, k * t + kk:k * t + kk + 1],
                                None,
                                op0=mybir.AluOpType.is_equal)
                    psA = pp.tile([mw, sup * segcap], f32, name=f"psA{last}",
                                  tag="pA")
                    for i in range(sup):
                        for kk in range(k):
                            nc.tensor.matmul(
                                psA[:, i * segcap:(i + 1) * segcap],
                                lhsT=wrows[i][:, kk * mw:(kk + 1) * mw],
                                rhs=ohs[i][:, kk * segcap:(kk + 1) * segcap],
                                start=(kk == 0), stop=(kk == k - 1))
                    # normalization: reciprocal of denom row, bcast, multiply
                    denr = epool.tile([1, sup * segcap], f32, name=f"denr{last}")
                    nc.vector.reciprocal(denr[:], psA[mw - 1:mw, :])
                    psB = pp.tile([fdim, sup * segcap], f32, name=f"psB{last}",
                                  tag="pB")
                    nc.tensor.matmul(psB[:], lhsT=ones_sb[:, 0:fdim],
                                     rhs=denr[:],
                                     start=True, stop=True)
                    num_sb = epool.tile([fdim, sup * segcap], f32,
                                        name=f"num{last}")
                    nc.vector.tensor_copy(num_sb[:], psA[0:fdim, :])
                    hn = epool.tile([fdim, sup * segcap], f32, name=f"hn{last}")
                    nc.vector.tensor_tensor(out=hn[:], in0=num_sb[:],
                                            in1=psB[:],
                                            op=mybir.AluOpType.mult)
                    if not last:
                        h2r = epool.tile([fdim, sup * segcap], f32, name="h2r")
                        nc.scalar.activation(h2r[:], hn[:],
                                             mybir.ActivationFunctionType.Relu,
                                             bias=b1_sb[:])
                        psC = pp.tile([c + 2, sup * segcap], f32, name="psC",
                                      tag="pC")
                        nc.tensor.matmul(psC[:], lhsT=w2aug_sb[:], rhs=h2r[:],
                                         start=True, stop=True)
                        ow = c + 2
                        fin_t = epool.tile([ow, sup * segcap], f32, name="fin1")
                        nc.vector.tensor_copy(fin_t[:], psC[:])
                    else:
                        ow = c
                        fin_t = epool.tile([ow, sup * segcap], f32, name="fin2")
                        nc.vector.tensor_scalar_add(fin_t[:], hn[:], b2_sb[:])
                    psD = pp.tile([128, k * ow], f32, name=f"psD{last}", tag="pD")
                    for q in range(k):
                        nc.tensor.transpose(
                            psD[:, q * ow:(q + 1) * ow],
                            in_=fin_t[:, q * 128:(q + 1) * 128],
                            identity=ident[0:ow, 0:ow])
                    orows = epool.tile([128, k * ow], f32, name=f"orows{last}")
                    nc.vector.tensor_copy(orows[:], psD[:])
                    for q in range(k):
                        nc.gpsimd.indirect_dma_start(
                            out=out_shard[:],
                            out_offset=bass.IndirectOffsetOnAxis(
                                ap=scat_sb[:, k * S + q:k * S + q + 1], axis=0),
                            in_=orows[:, q * ow:(q + 1) * ow], in_offset=None,
                            bounds_check=bound_reg, oob_is_err=False,
                        )

            edge_layer(h1f, h, g2s, last=False)
            nc.gpsimd.collective_compute(
                "AllGather", mybir.AluOpType.bypass,
                replica_groups=[list(range(N_CORES))],
                ins=[g2s[:]], outs=[g2f[:]],
            )
            edge_layer(g2f, c, out2, last=True)

    _compress_deps(nc)
    nc.compile()
    return nc


# ------------------------------------------------------------------ interface
def make_inmaps(inputs, cfg):
    x = np.ascontiguousarray(np.asarray(inputs["x"], np.float32))
    W1 = np.asarray(inputs["W1"], np.float32)
    as1 = np.asarray(inputs["att_src1"], np.float32)
    ad1 = np.asarray(inputs["att_dst1"], np.float32)
    b1 = np.asarray(inputs["b1"], np.float32)
    W2 = np.asarray(inputs["W2"], np.float32)
    as2 = np.asarray(inputs["att_src2"], np.float32)
    ad2 = np.asarray(inputs["att_dst2"], np.float32)
    b2 = np.asarray(inputs["b2"], np.float32)
    cores, T = preprocess(np.asarray(inputs["edge_index"]), cfg)
    w1aug = np.concatenate([W1, (W1 @ as1)[:, None], (W1 @ ad1)[:, None]], 1)
    w2aug = np.concatenate([W2, (W2 @ as2)[:, None], (W2 @ ad2)[:, None]], 1)
    nshard = cfg["nshard"]
    in_maps = []
    for cidx in range(N_CORES):
        xs = x[cidx * nshard:(cidx + 1) * nshard]  # [nshard, fin]
        in_maps.append(dict(
            xT=np.ascontiguousarray(xs.T),
            w1aug=np.ascontiguousarray(w1aug),
            w2aug=np.ascontiguousarray(w2aug),
            b1=np.ascontiguousarray(b1[:, None]),
            b2=np.ascontiguousarray(b2[:, None]),
            srcg=cores[cidx]["srcg"],
            dstg=cores[cidx]["dstg"],
            segid=cores[cidx]["segid"],
            scat=cores[cidx]["scat"],
        ))
    return in_maps, T


def kernel(**inputs):
    from concourse import bass_utils

    cfg = dict(DEF_CFG)
    in_maps, T = make_inmaps(inputs, cfg)
    nc = build_program(cfg, T)
    res = bass_utils.run_bass_kernel_spmd(
        nc, in_maps, core_ids=list(range(N_CORES)))
    out = np.concatenate([res.results[c]["out2"] for c in range(N_CORES)], 0)
    return out.astype(np.float32)


# revision 26
# speedup vs baseline: 1.4753x; 1.4753x over previous
"""Two-layer single-head GAT (PyG GATConv semantics) on 8 Trainium2 NeuronCores.

v2.5 (dst-sharded edge-parallel, bf16, a_dst-gather-free attention):
  * Host: add self-loops, sort edges by destination, shard destinations
    across 8 cores (12500 each), pack into 512-edge tiles (<=32 dst
    segments per tile), 16 tiles per super-tile.
  * Tables are bf16 rows [feat | a_src | a_dst] (stride 66 / 42).
  * Per 128-edge chunk, source rows are fetched with one [128,1]-index
    indirect DMA (the only indirect form real HW supports: one descriptor
    per partition reading a contiguous row).
  * The per-edge a_dst gather is ELIMINATED: each super-tile's destinations
    span <=512 consecutive nodes, so a compact per-(chunk,segment) a_dst
    row (adrow, built once per layer with ~14 indirect DMAs from a compact
    a_dst column) is partition-broadcast and the full per-(edge,segment)
    logit matrix esB = a_src[e] + a_dst[s] is formed with wide
    stride-0-broadcast DVE ops; p = exp(leakyrelu(esB)) masked by the
    one-hot (iota == segid) yields the p-scaled one-hot "phs" exactly.
  * Aggregation matmuls are bf16 (full PE rate).  Layer 1 normalizes via
    reciprocal + partition_broadcast, applies relu+bias and the
    W2aug projection, transposes, scatters bf16 rows into g2s, and dumps
    the a_dst2 row into a compact slot-order table for layer 2's adrow.
  * Layer 2 aggregates segment-major [32, feat] (denominators per
    partition: cheap normalization, no transposes) and writes slot-order
    f32 staging to DRAM; the host applies the static slot->node permutation
    during assembly.
"""

import numpy as np

N_NODES = 100000
N_CORES = 8
F_IN, H, C = 128, 64, 40

TILE_E = 512            # edges per edge-tile
KPART = TILE_E // 128   # 4 edge chunks per tile
SEGCAP = 32             # destination-segment slots per edge-tile
SUPER = 16              # edge-tiles per super-tile
DUMMY_SEG = 99.0        # segid for padding edges: matches no one-hot column
SENT = 1 << 24          # scatter index sentinel (> bounds_check -> skipped)

DEF_CFG = dict(
    n=N_NODES, nshard=N_NODES // N_CORES, fin=F_IN, h=H, c=C,
    tile_e=TILE_E, k=KPART, segcap=SEGCAP, sup=SUPER, ntw=500,
)


def _bf16(a):
    import ml_dtypes
    return np.ascontiguousarray(np.asarray(a, np.float32).astype(ml_dtypes.bfloat16))


# ----------------------------------------------------------------- host prep
def _pack_core(src_c, dst_c, base, cfg):
    nshard, tile_e, segcap = cfg["nshard"], cfg["tile_e"], cfg["segcap"]
    counts = np.bincount(dst_c - base, minlength=nshard)
    assert counts.min() >= 1 and counts.max() <= tile_e
    cum = np.concatenate([[0], np.cumsum(counts)])
    tiles = []
    i = 0
    while i < nshard:
        j = int(np.searchsorted(cum, cum[i] + tile_e, side="right")) - 1
        j = min(j, i + segcap, nshard)
        assert j > i
        tiles.append((i, j))
        i = j
    T = len(tiles)
    src_g = np.zeros((T, tile_e), np.int32)
    segid = np.full((T, tile_e), DUMMY_SEG, np.float32)
    dstseg = np.full((T, segcap), -1, np.int64)
    for t, (i, j) in enumerate(tiles):
        e0, e1 = int(cum[i]), int(cum[j])
        nE = e1 - e0
        src_g[t, :nE] = src_c[e0:e1]
        segid[t, :nE] = (dst_c[e0:e1] - base - i).astype(np.float32)
        dstseg[t, : j - i] = np.arange(i, j)
    tstart = np.array([i for (i, j) in tiles], np.int32)
    return src_g, segid, dstseg, tstart


def _edge_layout(arr_t):  # [T, tile_e] -> [128, T*K]: (t, k*128+p) -> [p, K*t+k]
    T, tile_e = arr_t.shape
    k = tile_e // 128
    return np.ascontiguousarray(
        arr_t.reshape(T, k, 128).transpose(2, 0, 1).reshape(128, T * k)
    )


def preprocess(edge_index, cfg):
    n, nshard, sup, segcap, k = (cfg["n"], cfg["nshard"], cfg["sup"],
                                 cfg["segcap"], cfg["k"])
    src = np.asarray(edge_index[0]).astype(np.int64)
    dst = np.asarray(edge_index[1]).astype(np.int64)
    loop = np.arange(n, dtype=np.int64)
    src = np.concatenate([src, loop])
    dst = np.concatenate([dst, loop])
    order = np.argsort(dst, kind="stable")
    src, dst = src[order], dst[order]
    bounds = np.searchsorted(dst, np.arange(N_CORES + 1) * nshard)
    packed = [
        _pack_core(src[bounds[c]: bounds[c + 1]], dst[bounds[c]: bounds[c + 1]],
                   c * nshard, cfg)
        for c in range(N_CORES)
    ]
    Tmax = max(p[0].shape[0] for p in packed)
    Tmax = -(-Tmax // sup) * sup
    cores = []
    for c, (src_g, segid, dstseg, tstart) in enumerate(packed):
        pad = Tmax - src_g.shape[0]
        src_g = np.pad(src_g, ((0, pad), (0, 0)))
        segid = np.pad(segid, ((0, pad), (0, 0)), constant_values=DUMMY_SEG)
        dstseg = np.pad(dstseg, ((0, pad), (0, 0)), constant_values=-1)
        tstart = np.pad(tstart, (0, pad))
        segs = dstseg.reshape(Tmax // sup, sup * segcap)
        scat = np.where(segs >= 0, segs, SENT).astype(np.int32)
        scat = _edge_layout(scat)  # [128, (T//sup)*4]
        # per-chunk tile starts (adrow gather indices): chunk cc -> tile cc//k
        ts1 = np.repeat(tstart, k).astype(np.int32)            # adc1 offsets
        ts2 = (np.repeat(np.arange(Tmax), k) * segcap).astype(np.int32)
        NTS = -(-Tmax * k // 128)
        ts1 = np.pad(ts1, (0, NTS * 128 - len(ts1)))
        ts2 = np.pad(ts2, (0, NTS * 128 - len(ts2)))
        cores.append(dict(
            srcg=_edge_layout(src_g),
            segid=_edge_layout(segid).astype(np.float32),
            scat=scat,
            dstseg=dstseg,
            tstart1=np.ascontiguousarray(ts1.reshape(NTS, 128).T),
            tstart2=np.ascontiguousarray(ts2.reshape(NTS, 128).T),
        ))
    return cores, Tmax


def _compress_deps(nc):
    """Drop redundant sync dependencies (walrus wait-slot pressure)."""
    f = nc.m.functions[0]

    def all_insts(blk):
        for i in blk.instructions:
            yield i
        for sb in getattr(blk, "blocks", []) or []:
            yield from all_insts(sb)

    insts = [i for b in f.blocks for i in all_insts(b)]
    pos = {i.name: p for p, i in enumerate(insts)}
    by_name = {i.name: i for i in insts}

    def stream_key(p):
        tname = type(p).__name__
        if tname == "InstCollectiveCompute":
            return None
        if tname == "InstDMACopy":
            return ("dma", str(getattr(p, "queue", "")), str(p.engine))
        return ("eng", str(p.engine))

    for i in insts:
        deps = list(i.sync_dependency_names())
        if len(deps) <= 2:
            continue
        best: dict = {}
        keep = []
        for d in deps:
            p = by_name.get(d)
            if p is None:
                keep.append(d)
                continue
            kk = stream_key(p)
            if kk is None:
                keep.append(d)
                continue
            cur = best.get(kk)
            if cur is None or pos[d] > pos[cur]:
                best[kk] = d
        keep += list(best.values())
        for d in deps:
            if d not in keep:
                i.try_remove_dependency(d)


# ------------------------------------------------------------- device program
def build_program(cfg, T):
    import concourse.bass as bass
    import concourse.bacc as bacc
    import concourse.mybir as mybir
    import concourse.tile as tile
    from concourse.masks import make_identity

    f32 = mybir.dt.float32
    bf16 = mybir.dt.bfloat16
    i32 = mybir.dt.int32
    AF = mybir.ActivationFunctionType
    OP = mybir.AluOpType
    nshard, fin, h, c = cfg["nshard"], cfg["fin"], cfg["h"], cfg["c"]
    k, segcap, sup, ntw = cfg["k"], cfg["segcap"], cfg["sup"], cfg["ntw"]
    n = cfg["n"]
    nsup = T // sup
    cps = sup * k            # 64 chunks per super-tile
    st1, st2 = h + 2, c + 2  # table row strides (66 / 42)
    rw1, rw2 = h + 1, c + 1  # row widths up to & incl a_src (65 / 41)
    qw = ntw // 4
    NTS = -(-T * k // 128)   # adrow gather call count

    nc = bacc.Bacc(
        "TRN2", target_bir_lowering=False, debug=False,
        enable_asserts=False, num_devices=N_CORES,
    )

    xT = nc.dram_tensor("xT", [fin, nshard], bf16, kind="ExternalInput").ap()
    w1aug = nc.dram_tensor("w1aug", [fin, st1], bf16, kind="ExternalInput").ap()
    w2aug = nc.dram_tensor("w2aug", [h, st2], bf16, kind="ExternalInput").ap()
    b1 = nc.dram_tensor("b1", [h, 1], f32, kind="ExternalInput").ap()
    b2r = nc.dram_tensor("b2r", [1, sup * c], f32, kind="ExternalInput").ap()
    iotaI = nc.dram_tensor("iotaI", [128, segcap], bf16,
                           kind="ExternalInput").ap()
    srcg = nc.dram_tensor("srcg", [128, T * k], i32, kind="ExternalInput").ap()
    segid = nc.dram_tensor("segid", [128, T * k], f32,
                           kind="ExternalInput").ap()
    scat = nc.dram_tensor("scat", [128, nsup * k], i32,
                          kind="ExternalInput").ap()
    tstart1 = nc.dram_tensor("tstart1", [128, NTS], i32,
                             kind="ExternalInput").ap()
    tstart2 = nc.dram_tensor("tstart2", [128, NTS], i32,
                             kind="ExternalInput").ap()
    out2 = nc.dram_tensor("out2", [nsup * segcap, sup * c], f32,
                          kind="ExternalOutput").ap()

    with tile.TileContext(nc) as tc:
        with (
            tc.tile_pool(name="consts", bufs=1) as cpool,
            tc.tile_pool(name="work", bufs=2) as wpool,
            tc.tile_pool(name="epil", bufs=3) as epool,
            tc.tile_pool(name="psum", bufs=2, space="PSUM") as pp,
            tc.tile_pool(name="dram", bufs=1, space="DRAM") as dpool,
        ):
            # ---- constants
            w1aug_sb = cpool.tile([fin, st1], bf16, name="w1aug_sb")
            nc.sync.dma_start(w1aug_sb[:], w1aug)
            w2aug_sb = cpool.tile([h, st2], bf16, name="w2aug_sb")
            nc.sync.dma_start(w2aug_sb[:], w2aug)
            b1_sb = cpool.tile([h, 1], f32, name="b1_sb")
            nc.sync.dma_start(b1_sb[:], b1)
            b2r_sb = cpool.tile([1, sup * c], f32, name="b2r_sb")
            nc.sync.dma_start(b2r_sb[:], b2r)
            b2b = cpool.tile([segcap, sup * c], f32, name="b2b")
            nc.gpsimd.partition_broadcast(b2b[:], b2r_sb[:])
            identB = cpool.tile([128, 128], bf16, name="identB")
            make_identity(nc, identB[:])
            iota_bf = cpool.tile([128, segcap], bf16, name="iota_bf")
            nc.sync.dma_start(iota_bf[:], iotaI)
            srcg_sb = cpool.tile([128, T * k], i32, name="srcg_sb")
            nc.sync.dma_start(srcg_sb[:], srcg)
            segid_sb = cpool.tile([128, T * k], f32, name="segid_sb")
            nc.sync.dma_start(segid_sb[:], segid)
            scat_sb = cpool.tile([128, nsup * k], i32, name="scat_sb")
            nc.sync.dma_start(scat_sb[:], scat)
            ts1_sb = cpool.tile([128, NTS], i32, name="ts1_sb")
            nc.sync.dma_start(ts1_sb[:], tstart1)
            ts2_sb = cpool.tile([128, NTS], i32, name="ts2_sb")
            nc.sync.dma_start(ts2_sb[:], tstart2)

            bound_reg = nc.gpsimd.to_reg(nshard - 1)

            # ---- internal DRAM tables (bf16)
            h1s = dpool.tile([nshard, st1], bf16, name="h1s")
            h1f = dpool.tile([n, st1], bf16, name="h1f", addr_space="Shared")
            g2s = dpool.tile([nshard, st2], bf16, name="g2s")
            g2f = dpool.tile([n, st2], bf16, name="g2f", addr_space="Shared")
            adc1 = dpool.tile([nshard + segcap, 1], bf16, name="adc1")
            adp2 = dpool.tile([(T + sup) * segcap, 1], bf16, name="adp2")
            adrD = [dpool.tile([NTS * 128, segcap], bf16,
                               name=f"adrD{i}") for i in range(2)]

            # ---- phase 0: h1aug shard + compact a_dst column
            for nt in range(nshard // ntw):
                o = nt * ntw
                xt = epool.tile([fin, ntw], bf16, name="xt")
                nc.sync.dma_start(xt[:], xT[:, o:o + ntw])
                psH = pp.tile([st1, ntw], f32, name="psH", tag="pA")
                nc.tensor.matmul(psH[:], lhsT=w1aug_sb[:], rhs=xt[:],
                                 start=True, stop=True)
                h1t = epool.tile([st1, ntw], bf16, name="h1t")
                nc.vector.tensor_copy(h1t[:], psH[:])
                psT = pp.tile([qw, 4 * st1], bf16, name="psT", tag="pD")
                for q in range(4):
                    nc.tensor.transpose(
                        psT[:, q * st1:(q + 1) * st1],
                        in_=h1t[:, q * qw:(q + 1) * qw],
                        identity=identB[0:st1, 0:st1],
                    )
                h1r = epool.tile([qw, 4 * st1], bf16, name="h1r")
                nc.scalar.activation(h1r[:], psT[:], AF.Copy)
                for q in range(4):
                    nc.sync.dma_start(
                        h1s[o + q * qw:o + (q + 1) * qw, :],
                        h1r[:, q * st1:(q + 1) * st1],
                    )
                    nc.sync.dma_start(
                        adc1[o + q * qw:o + (q + 1) * qw, :],
                        h1r[:, q * st1 + st1 - 1:(q + 1) * st1],
                    )

            nc.gpsimd.collective_compute(
                "AllGather", mybir.AluOpType.bypass,
                replica_groups=[list(range(N_CORES))],
                ins=[h1s[:]], outs=[h1f[:]],
            )

            def build_adrow(src_dram, ts_sb, dst_dram):
                """adrow chunk g*128+p holds segcap consecutive a_dst values
                starting at ts[g*128+p] of the compact column src_dram."""
                for g in range(NTS):
                    adpc = epool.tile([128, segcap], bf16, name="adpc")
                    nc.gpsimd.indirect_dma_start(
                        out=adpc[:], out_offset=None,
                        in_=src_dram[:],
                        in_offset=bass.IndirectOffsetOnAxis(
                            ap=ts_sb[:, g:g + 1], axis=0),
                        element_offset=0,
                    )
                    nc.sync.dma_start(
                        dst_dram[g * 128:(g + 1) * 128, :], adpc[:])

            build_adrow(adc1, ts1_sb, adrD[0])

            # ---- edge layer
            def edge_layer(tab, adr_dram, rw, st, last):
                lay = int(last)
                for S in range(nsup):
                    c0 = cps * S
                    rows = wpool.tile([128, cps * st], bf16,
                                      name=f"rows{lay}")
                    for cc in range(cps):
                        nc.gpsimd.indirect_dma_start(
                            out=rows[:, st * cc:st * (cc + 1)],
                            out_offset=None,
                            in_=tab[:],
                            in_offset=bass.IndirectOffsetOnAxis(
                                ap=srcg_sb[:, c0 + cc:c0 + cc + 1], axis=0),
                            element_offset=0,
                        )
                    adrow = wpool.tile([1, cps * segcap], bf16,
                                       name=f"adr{lay}")
                    nc.sync.dma_start(
                        adrow[:],
                        adr_dram[c0:c0 + cps, :].rearrange(
                            "(o a) b -> o (a b)", o=1))
                    adB = wpool.tile([128, cps * segcap], bf16,
                                     name=f"adB{lay}")
                    nc.gpsimd.partition_broadcast(adB[:], adrow[:])
                    rv = rows[:].rearrange("p (m f) -> p m f", f=st)
                    asb = wpool.tile([128, cps], bf16, name=f"as{lay}")
                    nc.vector.tensor_scalar(asb[:], rv[:, :, rw - 1],
                                            15.0, -15.0,
                                            op0=OP.min, op1=OP.max)
                    nc.vector.memset(rv[:, :, rw - 1], 1.0)
                    av = asb[:]
                    asX = bass.AP(av.tensor, av.offset,
                                  [list(av.ap[0]), list(av.ap[1]),
                                   [0, segcap]])
                    esB = wpool.tile([128, cps * segcap], bf16,
                                     name=f"esB{lay}")
                    nc.vector.tensor_tensor(out=esB[:], in0=asX, in1=adB[:],
                                            op=OP.add)
                    e2B = wpool.tile([128, cps * segcap], bf16,
                                     name=f"e2B{lay}")
                    nc.vector.tensor_scalar_mul(e2B[:], esB[:], 0.2)
                    nc.vector.tensor_tensor(out=esB[:], in0=esB[:], in1=e2B[:],
                                            op=OP.max)
                    nc.vector.tensor_scalar(esB[:], esB[:], 15.0, -15.0,
                                            op0=OP.min, op1=OP.max)
                    pB = wpool.tile([128, cps * segcap], bf16, name=f"pB{lay}")
                    nc.scalar.activation(pB[:], esB[:], AF.Exp)
                    it = iota_bf[:]
                    iotaX = bass.AP(it.tensor, it.offset,
                                    [list(it.ap[0]), [0, cps], list(it.ap[1])])
                    sg = segid_sb[:, c0:c0 + cps]
                    segX = bass.AP(sg.tensor, sg.offset,
                                   [list(sg.ap[0]), list(sg.ap[1]),
                                    [0, segcap]])
                    ohB = wpool.tile([128, cps * segcap], bf16,
                                     name=f"oh{lay}")
                    nc.vector.tensor_tensor(out=ohB[:], in0=iotaX, in1=segX,
                                            op=OP.is_equal)
                    phs = wpool.tile([128, cps * segcap], bf16,
                                     name=f"phs{lay}")
                    nc.vector.tensor_tensor(out=phs[:], in0=pB[:], in1=ohB[:],
                                            op=OP.mult)
                    if not last:
                        psA = pp.tile([rw, sup * segcap], f32, name="psA",
                                      tag="pA")
                        for i in range(sup):
                            for kk in range(k):
                                cc = k * i + kk
                                nc.tensor.matmul(
                                    psA[:, segcap * i:segcap * (i + 1)],
                                    lhsT=rows[:, st * cc:st * cc + rw],
                                    rhs=phs[:, segcap * cc:segcap * (cc + 1)],
                                    start=(kk == 0), stop=(kk == k - 1))
                        denr = epool.tile([1, sup * segcap], f32, name="denr")
                        nc.vector.reciprocal(
                            denr[:], psA[rw - 1:rw, :])
                        denb = epool.tile([1, sup * segcap], bf16, name="denb")
                        nc.scalar.activation(denb[:], denr[:], AF.Copy)
                        denbB = epool.tile([h, sup * segcap], bf16,
                                           name="denbB")
                        nc.gpsimd.partition_broadcast(denbB[:], denb[:])
                        hnb = epool.tile([h, sup * segcap], bf16, name="hnb")
                        nc.vector.tensor_tensor(out=hnb[:], in0=psA[0:h, :],
                                                in1=denbB[:], op=OP.mult)
                        h2r = epool.tile([h, sup * segcap], bf16, name="h2r")
                        nc.scalar.activation(h2r[:], hnb[:], AF.Relu,
                                             bias=b1_sb[:])
                        psC = pp.tile([st2, sup * segcap], f32, name="psC",
                                      tag="pC")
                        nc.tensor.matmul(psC[:], lhsT=w2aug_sb[:], rhs=h2r[:],
                                         start=True, stop=True)
                        fin1 = epool.tile([st2, sup * segcap], bf16,
                                          name="fin1")
                        nc.vector.tensor_copy(fin1[:], psC[:])
                        nc.sync.dma_start(
                            adp2[S * sup * segcap:(S + 1) * sup * segcap, :]
                            .rearrange("(o a) b -> o (a b)", o=1),
                            fin1[st2 - 1:st2, :])
                        psD = pp.tile([128, k * st2], bf16, name="psD",
                                      tag="pD")
                        for q in range(k):
                            nc.tensor.transpose(
                                psD[:, st2 * q:st2 * (q + 1)],
                                in_=fin1[:, 128 * q:128 * (q + 1)],
                                identity=identB[0:st2, 0:st2])
                        orows = epool.tile([128, k * st2], bf16, name="orows")
                        nc.scalar.activation(orows[:], psD[:], AF.Copy)
                        for q in range(k):
                            nc.gpsimd.indirect_dma_start(
                                out=g2s[:],
                                out_offset=bass.IndirectOffsetOnAxis(
                                    ap=scat_sb[:, k * S + q:k * S + q + 1],
                                    axis=0),
                                in_=orows[:, st2 * q:st2 * (q + 1)],
                                in_offset=None,
                                bounds_check=bound_reg, oob_is_err=False,
                            )
                    else:
                        half = sup // 2
                        psE = [pp.tile([segcap, half * rw], f32,
                                       name=f"psE{j}", tag=t2)
                               for j, t2 in enumerate(("pA", "pC"))]
                        for i in range(sup):
                            ps = psE[i // half]
                            col = (i % half) * rw
                            for kk in range(k):
                                cc = k * i + kk
                                nc.tensor.matmul(
                                    ps[:, col:col + rw],
                                    lhsT=phs[:, segcap * cc:segcap * (cc + 1)],
                                    rhs=rows[:, st * cc:st * cc + rw],
                                    start=(kk == 0), stop=(kk == k - 1))
                        r2 = epool.tile([segcap, sup], f32, name="r2")
                        for j in range(2):
                            dv = psE[j][:].rearrange(
                                "p (m f) -> p m f", f=rw)[:, :, rw - 1]
                            nc.vector.reciprocal(
                                r2[:, half * j:half * (j + 1)], dv)
                        fin2 = epool.tile([segcap, sup * (rw - 1)], f32,
                                          name="fin2")
                        for i in range(sup):
                            ps = psE[i // half]
                            col = (i % half) * rw
                            nc.vector.tensor_scalar_mul(
                                fin2[:, (rw - 1) * i:(rw - 1) * (i + 1)],
                                ps[:, col:col + rw - 1], r2[:, i:i + 1])
                        nc.vector.tensor_tensor(out=fin2[:], in0=fin2[:],
                                                in1=b2b[:], op=OP.add)
                        nc.sync.dma_start(
                            out2[S * segcap:(S + 1) * segcap, :], fin2[:])

            edge_layer(h1f, adrD[0], rw1, st1, last=False)
            build_adrow(adp2, ts2_sb, adrD[1])
            nc.gpsimd.collective_compute(
                "AllGather", mybir.AluOpType.bypass,
                replica_groups=[list(range(N_CORES))],
                ins=[g2s[:]], outs=[g2f[:]],
            )
            edge_layer(g2f, adrD[1], rw2, st2, last=True)

    _compress_deps(nc)
    nc.compile()
    return nc


# ------------------------------------------------------------------ interface
def make_inmaps(inputs, cfg):
    x = np.asarray(inputs["x"], np.float32)
    W1 = np.asarray(inputs["W1"], np.float32)
    as1 = np.asarray(inputs["att_src1"], np.float32)
    ad1 = np.asarray(inputs["att_dst1"], np.float32)
    b1v = np.asarray(inputs["b1"], np.float32)
    W2 = np.asarray(inputs["W2"], np.float32)
    as2 = np.asarray(inputs["att_src2"], np.float32)
    ad2 = np.asarray(inputs["att_dst2"], np.float32)
    b2v = np.asarray(inputs["b2"], np.float32)
    cores, T = preprocess(np.asarray(inputs["edge_index"]), cfg)
    w1aug = np.concatenate([W1, (W1 @ as1)[:, None], (W1 @ ad1)[:, None]], 1)
    w2aug = np.concatenate([W2, (W2 @ as2)[:, None], (W2 @ ad2)[:, None]], 1)
    nshard, sup, segcap = cfg["nshard"], cfg["sup"], cfg["segcap"]
    b2row = np.tile(b2v, sup)[None, :]
    iota = np.tile(np.arange(segcap, dtype=np.float32), (128, 1))
    in_maps = []
    for cidx in range(N_CORES):
        co = cores[cidx]
        xs = x[cidx * nshard:(cidx + 1) * nshard]
        in_maps.append(dict(
            xT=_bf16(xs.T),
            w1aug=_bf16(w1aug),
            w2aug=_bf16(w2aug),
            b1=np.ascontiguousarray(b1v[:, None]),
            b2r=np.ascontiguousarray(b2row),
            iotaI=_bf16(iota),
            srcg=co["srcg"],
            segid=co["segid"],
            scat=co["scat"],
            tstart1=co["tstart1"],
            tstart2=co["tstart2"],
        ))
    return in_maps, T, cores


def _assemble(cores, results, cfg):
    nshard, sup, segcap, c = (cfg["nshard"], cfg["sup"], cfg["segcap"],
                              cfg["c"])
    out = np.zeros((N_CORES * nshard, c), np.float32)
    for cidx in range(N_CORES):
        co = cores[cidx]
        dstseg = co["dstseg"]                       # [Tmax, segcap]
        Tm = dstseg.shape[0]
        stg = np.asarray(results[cidx]["out2"], np.float32).reshape(
            Tm // sup, segcap, sup, c)              # [S, p, t, c]
        S_i = np.arange(Tm) // sup
        t_i = np.arange(Tm) % sup
        for t in range(Tm):
            m = dstseg[t] >= 0
            if m.any():
                nodes = dstseg[t][m] + cidx * nshard
                out[nodes] = stg[S_i[t], m, t_i[t], :]
    return out


def kernel(**inputs):
    from concourse import bass_utils

    cfg = dict(DEF_CFG)
    in_maps, T, cores = make_inmaps(inputs, cfg)
    nc = build_program(cfg, T)
    res = bass_utils.run_bass_kernel_spmd(
        nc, in_maps, core_ids=list(range(N_CORES)))
    return _assemble(cores, res.results, cfg).astype(np.float32)


# revision 29
# speedup vs baseline: 1.6839x; 1.1413x over previous
"""Two-layer single-head GAT (PyG GATConv semantics) on 8 Trainium2 NeuronCores.

v2.5 (dst-sharded edge-parallel, bf16, a_dst-gather-free attention):
  * Host: add self-loops, sort edges by destination, shard destinations
    across 8 cores (12500 each), pack into 512-edge tiles (<=32 dst
    segments per tile), 16 tiles per super-tile.
  * Tables are bf16 rows [feat | a_src | a_dst] (stride 66 / 42).
  * Per 128-edge chunk, source rows are fetched with one [128,1]-index
    indirect DMA (the only indirect form real HW supports: one descriptor
    per partition reading a contiguous row).
  * The per-edge a_dst gather is ELIMINATED: each super-tile's destinations
    span <=512 consecutive nodes, so a compact per-(chunk,segment) a_dst
    row (adrow, built once per layer with ~14 indirect DMAs from a compact
    a_dst column) is partition-broadcast and the full per-(edge,segment)
    logit matrix esB = a_src[e] + a_dst[s] is formed with wide
    stride-0-broadcast DVE ops; p = exp(leakyrelu(esB)) masked by the
    one-hot (iota == segid) yields the p-scaled one-hot "phs" exactly.
  * Aggregation matmuls are bf16 (full PE rate).  Layer 1 normalizes via
    reciprocal + partition_broadcast, applies relu+bias and the
    W2aug projection, transposes, scatters bf16 rows into g2s, and dumps
    the a_dst2 row into a compact slot-order table for layer 2's adrow.
  * Layer 2 aggregates segment-major [32, feat] (denominators per
    partition: cheap normalization, no transposes) and writes slot-order
    f32 staging to DRAM; the host applies the static slot->node permutation
    during assembly.
"""

import numpy as np

N_NODES = 100000
N_CORES = 8
F_IN, H, C = 128, 64, 40

TILE_E = 512            # edges per edge-tile
KPART = TILE_E // 128   # 4 edge chunks per tile
SEGCAP = 32             # destination-segment slots per edge-tile
SUPER = 16              # edge-tiles per super-tile
DUMMY_SEG = 99.0        # segid for padding edges: matches no one-hot column
SENT = 1 << 24          # scatter index sentinel (> bounds_check -> skipped)

DEF_CFG = dict(
    n=N_NODES, nshard=N_NODES // N_CORES, fin=F_IN, h=H, c=C,
    tile_e=TILE_E, k=KPART, segcap=SEGCAP, sup=SUPER, ntw=500,
)


def _bf16(a):
    import ml_dtypes
    return np.ascontiguousarray(np.asarray(a, np.float32).astype(ml_dtypes.bfloat16))


# ----------------------------------------------------------------- host prep
def _pack_core(src_c, dst_c, base, cfg):
    nshard, tile_e, segcap = cfg["nshard"], cfg["tile_e"], cfg["segcap"]
    counts = np.bincount(dst_c - base, minlength=nshard)
    assert counts.min() >= 1 and counts.max() <= tile_e
    cum = np.concatenate([[0], np.cumsum(counts)])
    tiles = []
    i = 0
    while i < nshard:
        j = int(np.searchsorted(cum, cum[i] + tile_e, side="right")) - 1
        j = min(j, i + segcap, nshard)
        assert j > i
        tiles.append((i, j))
        i = j
    T = len(tiles)
    src_g = np.zeros((T, tile_e), np.int32)
    segid = np.full((T, tile_e), DUMMY_SEG, np.float32)
    dstseg = np.full((T, segcap), -1, np.int64)
    for t, (i, j) in enumerate(tiles):
        e0, e1 = int(cum[i]), int(cum[j])
        nE = e1 - e0
        src_g[t, :nE] = src_c[e0:e1]
        segid[t, :nE] = (dst_c[e0:e1] - base - i).astype(np.float32)
        dstseg[t, : j - i] = np.arange(i, j)
    tstart = np.array([i for (i, j) in tiles], np.int32)
    return src_g, segid, dstseg, tstart


def _edge_layout(arr_t):  # [T, tile_e] -> [128, T*K]: (t, k*128+p) -> [p, K*t+k]
    T, tile_e = arr_t.shape
    k = tile_e // 128
    return np.ascontiguousarray(
        arr_t.reshape(T, k, 128).transpose(2, 0, 1).reshape(128, T * k)
    )


def preprocess(edge_index, cfg):
    n, nshard, sup, segcap, k = (cfg["n"], cfg["nshard"], cfg["sup"],
                                 cfg["segcap"], cfg["k"])
    src = np.asarray(edge_index[0]).astype(np.int64)
    dst = np.asarray(edge_index[1]).astype(np.int64)
    loop = np.arange(n, dtype=np.int64)
    src = np.concatenate([src, loop])
    dst = np.concatenate([dst, loop])
    order = np.argsort(dst, kind="stable")
    src, dst = src[order], dst[order]
    bounds = np.searchsorted(dst, np.arange(N_CORES + 1) * nshard)
    packed = [
        _pack_core(src[bounds[c]: bounds[c + 1]], dst[bounds[c]: bounds[c + 1]],
                   c * nshard, cfg)
        for c in range(N_CORES)
    ]
    Tmax = max(p[0].shape[0] for p in packed)
    Tmax = -(-Tmax // sup) * sup
    cores = []
    for c, (src_g, segid, dstseg, tstart) in enumerate(packed):
        pad = Tmax - src_g.shape[0]
        src_g = np.pad(src_g, ((0, pad), (0, 0)))
        segid = np.pad(segid, ((0, pad), (0, 0)), constant_values=DUMMY_SEG)
        dstseg = np.pad(dstseg, ((0, pad), (0, 0)), constant_values=-1)
        tstart = np.pad(tstart, (0, pad))
        segs = dstseg.reshape(Tmax // sup, sup * segcap)
        scat = np.where(segs >= 0, segs, SENT).astype(np.int32)
        scat = _edge_layout(scat)  # [128, (T//sup)*4]
        # per-chunk tile starts (adrow gather indices): chunk cc -> tile cc//k
        ts1 = np.repeat(tstart, k).astype(np.int32)            # adc1 offsets
        ts2 = (np.repeat(np.arange(Tmax), k) * segcap).astype(np.int32)
        NTS = -(-Tmax * k // 128)
        ts1 = np.pad(ts1, (0, NTS * 128 - len(ts1)))
        ts2 = np.pad(ts2, (0, NTS * 128 - len(ts2)))
        cores.append(dict(
            srcg=_edge_layout(src_g),
            segid=_edge_layout(segid).astype(np.float32),
            scat=scat,
            dstseg=dstseg,
            tstart1=np.ascontiguousarray(ts1.reshape(NTS, 128).T),
            tstart2=np.ascontiguousarray(ts2.reshape(NTS, 128).T),
        ))
    return cores, Tmax


def _compress_deps(nc):
    """Drop redundant sync dependencies (walrus wait-slot pressure)."""
    f = nc.m.functions[0]

    def all_insts(blk):
        for i in blk.instructions:
            yield i
        for sb in getattr(blk, "blocks", []) or []:
            yield from all_insts(sb)

    insts = [i for b in f.blocks for i in all_insts(b)]
    pos = {i.name: p for p, i in enumerate(insts)}
    by_name = {i.name: i for i in insts}

    def stream_key(p):
        tname = type(p).__name__
        if tname == "InstCollectiveCompute":
            return None
        if tname == "InstDMACopy":
            return ("dma", str(getattr(p, "queue", "")), str(p.engine))
        return ("eng", str(p.engine))

    for i in insts:
        deps = list(i.sync_dependency_names())
        if len(deps) <= 2:
            continue
        best: dict = {}
        keep = []
        for d in deps:
            p = by_name.get(d)
            if p is None:
                keep.append(d)
                continue
            kk = stream_key(p)
            if kk is None:
                keep.append(d)
                continue
            cur = best.get(kk)
            if cur is None or pos[d] > pos[cur]:
                best[kk] = d
        keep += list(best.values())
        for d in deps:
            if d not in keep:
                i.try_remove_dependency(d)


# ------------------------------------------------------------- device program
def build_program(cfg, T):
    import concourse.bass as bass
    import concourse.bacc as bacc
    import concourse.mybir as mybir
    import concourse.tile as tile
    from concourse.masks import make_identity

    f32 = mybir.dt.float32
    bf16 = mybir.dt.bfloat16
    i32 = mybir.dt.int32
    AF = mybir.ActivationFunctionType
    OP = mybir.AluOpType
    nshard, fin, h, c = cfg["nshard"], cfg["fin"], cfg["h"], cfg["c"]
    k, segcap, sup, ntw = cfg["k"], cfg["segcap"], cfg["sup"], cfg["ntw"]
    n = cfg["n"]
    nsup = T // sup
    cps = sup * k            # 64 chunks per super-tile
    st1, st2 = h + 2, c + 2  # table row strides (66 / 42)
    rw1, rw2 = h + 1, c + 1  # row widths up to & incl a_src (65 / 41)
    qw = ntw // 4
    NTS = -(-T * k // 128)   # adrow gather call count

    nc = bacc.Bacc(
        "TRN2", target_bir_lowering=False, debug=False,
        enable_asserts=False, num_devices=N_CORES,
    )

    xT = nc.dram_tensor("xT", [fin, nshard], bf16, kind="ExternalInput").ap()
    w1aug = nc.dram_tensor("w1aug", [fin, st1], bf16, kind="ExternalInput").ap()
    w2aug = nc.dram_tensor("w2aug", [h, st2], bf16, kind="ExternalInput").ap()
    b1 = nc.dram_tensor("b1", [h, 1], f32, kind="ExternalInput").ap()
    b2r = nc.dram_tensor("b2r", [1, sup * c], f32, kind="ExternalInput").ap()
    iotaI = nc.dram_tensor("iotaI", [128, segcap], bf16,
                           kind="ExternalInput").ap()
    srcg = nc.dram_tensor("srcg", [128, T * k], i32, kind="ExternalInput").ap()
    segid = nc.dram_tensor("segid", [128, T * k], f32,
                           kind="ExternalInput").ap()
    scat = nc.dram_tensor("scat", [128, nsup * k], i32,
                          kind="ExternalInput").ap()
    tstart1 = nc.dram_tensor("tstart1", [128, NTS], i32,
                             kind="ExternalInput").ap()
    tstart2 = nc.dram_tensor("tstart2", [128, NTS], i32,
                             kind="ExternalInput").ap()
    out2 = nc.dram_tensor("out2", [nsup * segcap, sup * c], f32,
                          kind="ExternalOutput").ap()

    with tile.TileContext(nc) as tc:
        with (
            tc.tile_pool(name="consts", bufs=1) as cpool,
            tc.tile_pool(name="work", bufs=2) as wpool,
            tc.tile_pool(name="epil", bufs=3) as epool,
            tc.tile_pool(name="psum", bufs=2, space="PSUM") as pp,
            tc.tile_pool(name="dram", bufs=1, space="DRAM") as dpool,
        ):
            # ---- constants
            w1aug_sb = cpool.tile([fin, st1], bf16, name="w1aug_sb")
            nc.sync.dma_start(w1aug_sb[:], w1aug)
            w2aug_sb = cpool.tile([h, st2], bf16, name="w2aug_sb")
            nc.sync.dma_start(w2aug_sb[:], w2aug)
            b1_sb = cpool.tile([h, 1], f32, name="b1_sb")
            nc.sync.dma_start(b1_sb[:], b1)
            b2r_sb = cpool.tile([1, sup * c], f32, name="b2r_sb")
            nc.sync.dma_start(b2r_sb[:], b2r)
            b2b = cpool.tile([segcap, sup * c], f32, name="b2b")
            nc.gpsimd.partition_broadcast(b2b[:], b2r_sb[:])
            identB = cpool.tile([128, 128], bf16, name="identB")
            make_identity(nc, identB[:])
            iota_bf = cpool.tile([128, segcap], bf16, name="iota_bf")
            nc.sync.dma_start(iota_bf[:], iotaI)
            srcg_sb = cpool.tile([128, T * k], i32, name="srcg_sb")
            nc.sync.dma_start(srcg_sb[:], srcg)
            segid_sb = cpool.tile([128, T * k], f32, name="segid_sb")
            nc.sync.dma_start(segid_sb[:], segid)
            scat_sb = cpool.tile([128, nsup * k], i32, name="scat_sb")
            nc.sync.dma_start(scat_sb[:], scat)
            ts1_sb = cpool.tile([128, NTS], i32, name="ts1_sb")
            nc.sync.dma_start(ts1_sb[:], tstart1)
            ts2_sb = cpool.tile([128, NTS], i32, name="ts2_sb")
            nc.sync.dma_start(ts2_sb[:], tstart2)

            bound_reg = nc.gpsimd.to_reg(nshard - 1)

            # ---- internal DRAM tables (bf16)
            h1s = dpool.tile([nshard, st1], bf16, name="h1s")
            h1f = dpool.tile([n, st1], bf16, name="h1f", addr_space="Shared")
            g2s = dpool.tile([nshard, st2], bf16, name="g2s")
            g2f = dpool.tile([n, st2], bf16, name="g2f", addr_space="Shared")
            adc1 = dpool.tile([nshard + segcap, 1], bf16, name="adc1")
            adp2 = dpool.tile([(T + sup) * segcap, 1], bf16, name="adp2")
            adrD = [dpool.tile([NTS * 128, segcap], bf16,
                               name=f"adrD{i}") for i in range(2)]

            # ---- phase 0: h1aug shard + compact a_dst column
            for nt in range(nshard // ntw):
                o = nt * ntw
                xt = epool.tile([fin, ntw], bf16, name="xt")
                nc.sync.dma_start(xt[:], xT[:, o:o + ntw])
                psH = pp.tile([st1, ntw], f32, name="psH", tag="pA")
                nc.tensor.matmul(psH[:], lhsT=w1aug_sb[:], rhs=xt[:],
                                 start=True, stop=True)
                h1t = epool.tile([st1, ntw], bf16, name="h1t")
                nc.vector.tensor_copy(h1t[:], psH[:])
                psT = pp.tile([qw, 4 * st1], bf16, name="psT", tag="pD")
                for q in range(4):
                    nc.tensor.transpose(
                        psT[:, q * st1:(q + 1) * st1],
                        in_=h1t[:, q * qw:(q + 1) * qw],
                        identity=identB[0:st1, 0:st1],
                    )
                h1r = epool.tile([qw, 4 * st1], bf16, name="h1r")
                nc.scalar.activation(h1r[:], psT[:], AF.Copy)
                for q in range(4):
                    nc.sync.dma_start(
                        h1s[o + q * qw:o + (q + 1) * qw, :],
                        h1r[:, q * st1:(q + 1) * st1],
                    )
                    nc.sync.dma_start(
                        adc1[o + q * qw:o + (q + 1) * qw, :],
                        h1r[:, q * st1 + st1 - 1:(q + 1) * st1],
                    )

            nc.gpsimd.collective_compute(
                "AllGather", mybir.AluOpType.bypass,
                replica_groups=[list(range(N_CORES))],
                ins=[h1s[:]], outs=[h1f[:]],
            )

            def build_adrow(src_dram, ts_sb, dst_dram):
                """adrow chunk g*128+p holds segcap consecutive a_dst values
                starting at ts[g*128+p] of the compact column src_dram."""
                for g in range(NTS):
                    adpc = epool.tile([128, segcap], bf16, name="adpc")
                    nc.gpsimd.indirect_dma_start(
                        out=adpc[:], out_offset=None,
                        in_=src_dram[:],
                        in_offset=bass.IndirectOffsetOnAxis(
                            ap=ts_sb[:, g:g + 1], axis=0),
                        element_offset=0,
                    )
                    nc.sync.dma_start(
                        dst_dram[g * 128:(g + 1) * 128, :], adpc[:])

            build_adrow(adc1, ts1_sb, adrD[0])

            # ---- edge layer
            def edge_layer(tab, adr_dram, rw, st, last):
                lay = int(last)
                for S in range(nsup):
                    c0 = cps * S
                    rows = wpool.tile([128, cps * st], bf16,
                                      name=f"rows{lay}")
                    for cc in range(cps):
                        nc.gpsimd.indirect_dma_start(
                            out=rows[:, st * cc:st * (cc + 1)],
                            out_offset=None,
                            in_=tab[:],
                            in_offset=bass.IndirectOffsetOnAxis(
                                ap=srcg_sb[:, c0 + cc:c0 + cc + 1], axis=0),
                            element_offset=0,
                        )
                    adrow = wpool.tile([1, cps * segcap], bf16,
                                       name=f"adr{lay}")
                    nc.sync.dma_start(
                        adrow[:],
                        adr_dram[c0:c0 + cps, :].rearrange(
                            "(o a) b -> o (a b)", o=1))
                    adB = wpool.tile([128, cps * segcap], bf16,
                                     name=f"adB{lay}")
                    nc.gpsimd.partition_broadcast(adB[:], adrow[:])
                    rv = rows[:].rearrange("p (m f) -> p m f", f=st)
                    asb = wpool.tile([128, cps], bf16, name=f"as{lay}")
                    nc.vector.tensor_scalar(asb[:], rv[:, :, rw - 1],
                                            15.0, -15.0,
                                            op0=OP.min, op1=OP.max)
                    nc.vector.memset(rv[:, :, rw - 1], 1.0)
                    av = asb[:]
                    asX = bass.AP(av.tensor, av.offset,
                                  [list(av.ap[0]), list(av.ap[1]),
                                   [0, segcap]])
                    esB = wpool.tile([128, cps * segcap], bf16,
                                     name=f"esB{lay}")
                    nc.vector.tensor_tensor(out=esB[:], in0=asX, in1=adB[:],
                                            op=OP.add)
                    e2B = wpool.tile([128, cps * segcap], bf16,
                                     name=f"e2B{lay}")
                    nc.vector.tensor_scalar_mul(e2B[:], esB[:], 0.2)
                    nc.vector.tensor_tensor(out=esB[:], in0=esB[:], in1=e2B[:],
                                            op=OP.max)
                    nc.vector.tensor_scalar(esB[:], esB[:], 15.0, -15.0,
                                            op0=OP.min, op1=OP.max)
                    pB = wpool.tile([128, cps * segcap], bf16, name=f"pB{lay}")
                    nc.scalar.activation(pB[:], esB[:], AF.Exp)
                    it = iota_bf[:]
                    iotaX = bass.AP(it.tensor, it.offset,
                                    [list(it.ap[0]), [0, cps], list(it.ap[1])])
                    sg = segid_sb[:, c0:c0 + cps]
                    segX = bass.AP(sg.tensor, sg.offset,
                                   [list(sg.ap[0]), list(sg.ap[1]),
                                    [0, segcap]])
                    ohB = wpool.tile([128, cps * segcap], bf16,
                                     name=f"oh{lay}")
                    nc.vector.tensor_tensor(out=ohB[:], in0=iotaX, in1=segX,
                                            op=OP.is_equal)
                    phs = wpool.tile([128, cps * segcap], bf16,
                                     name=f"phs{lay}")
                    nc.vector.tensor_tensor(out=phs[:], in0=pB[:], in1=ohB[:],
                                            op=OP.mult)
                    if not last:
                        psA = pp.tile([rw, sup * segcap], f32, name="psA",
                                      tag="pA")
                        for i in range(sup):
                            for kk in range(k):
                                cc = k * i + kk
                                nc.tensor.matmul(
                                    psA[:, segcap * i:segcap * (i + 1)],
                                    lhsT=rows[:, st * cc:st * cc + rw],
                                    rhs=phs[:, segcap * cc:segcap * (cc + 1)],
                                    start=(kk == 0), stop=(kk == k - 1))
                        denr = epool.tile([1, sup * segcap], f32, name="denr")
                        nc.vector.reciprocal(
                            denr[:], psA[rw - 1:rw, :])
                        denb = epool.tile([1, sup * segcap], bf16, name="denb")
                        nc.scalar.activation(denb[:], denr[:], AF.Copy)
                        denbB = epool.tile([h, sup * segcap], bf16,
                                           name="denbB")
                        nc.gpsimd.partition_broadcast(denbB[:], denb[:])
                        hnb = epool.tile([h, sup * segcap], bf16, name="hnb")
                        nc.vector.tensor_tensor(out=hnb[:], in0=psA[0:h, :],
                                                in1=denbB[:], op=OP.mult)
                        h2r = epool.tile([h, sup * segcap], bf16, name="h2r")
                        nc.scalar.activation(h2r[:], hnb[:], AF.Relu,
                                             bias=b1_sb[:])
                        psC = pp.tile([st2, sup * segcap], f32, name="psC",
                                      tag="pC")
                        nc.tensor.matmul(psC[:], lhsT=w2aug_sb[:], rhs=h2r[:],
                                         start=True, stop=True)
                        fin1 = epool.tile([st2, sup * segcap], bf16,
                                          name="fin1")
                        nc.vector.tensor_copy(fin1[:], psC[:])
                        nc.sync.dma_start(
                            adp2[S * sup * segcap:(S + 1) * sup * segcap, :]
                            .rearrange("(o a) b -> o (a b)", o=1),
                            fin1[st2 - 1:st2, :])
                        psD = pp.tile([128, k * st2], bf16, name="psD",
                                      tag="pD")
                        for q in range(k):
                            nc.tensor.transpose(
                                psD[:, st2 * q:st2 * (q + 1)],
                                in_=fin1[:, 128 * q:128 * (q + 1)],
                                identity=identB[0:st2, 0:st2])
                        orows = epool.tile([128, k * st2], bf16, name="orows")
                        nc.scalar.activation(orows[:], psD[:], AF.Copy)
                        for q in range(k):
                            nc.gpsimd.indirect_dma_start(
                                out=g2s[:],
                                out_offset=bass.IndirectOffsetOnAxis(
                                    ap=scat_sb[:, k * S + q:k * S + q + 1],
                                    axis=0),
                                in_=orows[:, st2 * q:st2 * (q + 1)],
                                in_offset=None,
                                bounds_check=bound_reg, oob_is_err=False,
                            )
                    else:
                        half = sup // 2
                        psE = [pp.tile([segcap, half * rw], f32,
                                       name=f"psE{j}", tag=t2)
                               for j, t2 in enumerate(("pA", "pC"))]
                        for i in range(sup):
                            ps = psE[i // half]
                            col = (i % half) * rw
                            for kk in range(k):
                                cc = k * i + kk
                                nc.tensor.matmul(
                                    ps[:, col:col + rw],
                                    lhsT=phs[:, segcap * cc:segcap * (cc + 1)],
                                    rhs=rows[:, st * cc:st * cc + rw],
                                    start=(kk == 0), stop=(kk == k - 1))
                        r2 = epool.tile([segcap, sup], f32, name="r2")
                        for j in range(2):
                            dv = psE[j][:].rearrange(
                                "p (m f) -> p m f", f=rw)[:, :, rw - 1]
                            nc.vector.reciprocal(
                                r2[:, half * j:half * (j + 1)], dv)
                        fin2 = epool.tile([segcap, sup * (rw - 1)], f32,
                                          name="fin2")
                        for i in range(sup):
                            ps = psE[i // half]
                            col = (i % half) * rw
                            nc.vector.tensor_scalar_mul(
                                fin2[:, (rw - 1) * i:(rw - 1) * (i + 1)],
                                ps[:, col:col + rw - 1], r2[:, i:i + 1])
                        nc.vector.tensor_tensor(out=fin2[:], in0=fin2[:],
                                                in1=b2b[:], op=OP.add)
                        nc.sync.dma_start(
                            out2[S * segcap:(S + 1) * segcap, :], fin2[:])

            edge_layer(h1f, adrD[0], rw1, st1, last=False)
            build_adrow(adp2, ts2_sb, adrD[1])
            nc.gpsimd.collective_compute(
                "AllGather", mybir.AluOpType.bypass,
                replica_groups=[list(range(N_CORES))],
                ins=[g2s[:]], outs=[g2f[:]],
            )
            edge_layer(g2f, adrD[1], rw2, st2, last=True)

    _compress_deps(nc)
    nc.compile()
    return nc


# ------------------------------------------------------------------ interface
def make_inmaps(inputs, cfg):
    x = np.asarray(inputs["x"], np.float32)
    W1 = np.asarray(inputs["W1"], np.float32)
    as1 = np.asarray(inputs["att_src1"], np.float32)
    ad1 = np.asarray(inputs["att_dst1"], np.float32)
    b1v = np.asarray(inputs["b1"], np.float32)
    W2 = np.asarray(inputs["W2"], np.float32)
    as2 = np.asarray(inputs["att_src2"], np.float32)
    ad2 = np.asarray(inputs["att_dst2"], np.float32)
    b2v = np.asarray(inputs["b2"], np.float32)
    cores, T = preprocess(np.asarray(inputs["edge_index"]), cfg)
    w1aug = np.concatenate([W1, (W1 @ as1)[:, None], (W1 @ ad1)[:, None]], 1)
    w2aug = np.concatenate([W2, (W2 @ as2)[:, None], (W2 @ ad2)[:, None]], 1)
    nshard, sup, segcap = cfg["nshard"], cfg["sup"], cfg["segcap"]
    b2row = np.tile(b2v, sup)[None, :]
    iota = np.tile(np.arange(segcap, dtype=np.float32), (128, 1))
    in_maps = []
    for cidx in range(N_CORES):
        co = cores[cidx]
        xs = x[cidx * nshard:(cidx + 1) * nshard]
        in_maps.append(dict(
            xT=_bf16(xs.T),
            w1aug=_bf16(w1aug),
            w2aug=_bf16(w2aug),
            b1=np.ascontiguousarray(b1v[:, None]),
            b2r=np.ascontiguousarray(b2row),
            iotaI=_bf16(iota),
            srcg=co["srcg"],
            segid=co["segid"],
            scat=co["scat"],
            tstart1=co["tstart1"],
            tstart2=co["tstart2"],
        ))
    return in_maps, T, cores


def _assemble(cores, results, cfg):
    nshard, sup, segcap, c = (cfg["nshard"], cfg["sup"], cfg["segcap"],
                              cfg["c"])
    out = np.zeros((N_CORES * nshard, c), np.float32)
    for cidx in range(N_CORES):
        co = cores[cidx]
        dstseg = co["dstseg"]                       # [Tmax, segcap]
        Tm = dstseg.shape[0]
        stg = np.asarray(results[cidx]["out2"], np.float32).reshape(
            Tm // sup, segcap, sup, c)              # [S, p, t, c]
        S_i = np.arange(Tm) // sup
        t_i = np.arange(Tm) % sup
        for t in range(Tm):
            m = dstseg[t] >= 0
            if m.any():
                nodes = dstseg[t][m] + cidx * nshard
                out[nodes] = stg[S_i[t], m, t_i[t], :]
    return out


def kernel(**inputs):
    from concourse import bass_utils

    cfg = dict(DEF_CFG)
    in_maps, T, cores = make_inmaps(inputs, cfg)
    nc = build_program(cfg, T)
    res = bass_utils.run_bass_kernel_spmd(
        nc, in_maps, core_ids=list(range(N_CORES)))
    return _assemble(cores, res.results, cfg).astype(np.float32)


# revision 30
# speedup vs baseline: 1.8671x; 1.1088x over previous
"""Two-layer single-head GAT (PyG GATConv semantics) on 8 Trainium2 NeuronCores.

v2.5 (dst-sharded edge-parallel, bf16, a_dst-gather-free attention):
  * Host: add self-loops, sort edges by destination, shard destinations
    across 8 cores (12500 each), pack into 512-edge tiles (<=32 dst
    segments per tile), 16 tiles per super-tile.
  * Tables are bf16 rows [feat | a_src | a_dst] (stride 66 / 42).
  * Per 128-edge chunk, source rows are fetched with one [128,1]-index
    indirect DMA (the only indirect form real HW supports: one descriptor
    per partition reading a contiguous row).
  * The per-edge a_dst gather is ELIMINATED: each super-tile's destinations
    span <=512 consecutive nodes, so a compact per-(chunk,segment) a_dst
    row (adrow, built once per layer with ~14 indirect DMAs from a compact
    a_dst column) is partition-broadcast and the full per-(edge,segment)
    logit matrix esB = a_src[e] + a_dst[s] is formed with wide
    stride-0-broadcast DVE ops; p = exp(leakyrelu(esB)) masked by the
    one-hot (iota == segid) yields the p-scaled one-hot "phs" exactly.
  * Aggregation matmuls are bf16 (full PE rate).  Layer 1 normalizes via
    reciprocal + partition_broadcast, applies relu+bias and the
    W2aug projection, transposes, scatters bf16 rows into g2s, and dumps
    the a_dst2 row into a compact slot-order table for layer 2's adrow.
  * Layer 2 aggregates segment-major [32, feat] (denominators per
    partition: cheap normalization, no transposes) and writes slot-order
    f32 staging to DRAM; the host applies the static slot->node permutation
    during assembly.
"""

import numpy as np

N_NODES = 100000
N_CORES = 8
F_IN, H, C = 128, 64, 40

TILE_E = 512            # edges per edge-tile
KPART = TILE_E // 128   # 4 edge chunks per tile
SEGCAP = 32             # destination-segment slots per edge-tile
SUPER = 16              # edge-tiles per super-tile
DUMMY_SEG = 99.0        # segid for padding edges: matches no one-hot column
SENT = 1 << 24          # scatter index sentinel (> bounds_check -> skipped)

DEF_CFG = dict(
    n=N_NODES, nshard=N_NODES // N_CORES, fin=F_IN, h=H, c=C,
    tile_e=TILE_E, k=KPART, segcap=SEGCAP, sup=SUPER, ntw=500,
)


def _bf16(a):
    import ml_dtypes
    return np.ascontiguousarray(np.asarray(a, np.float32).astype(ml_dtypes.bfloat16))


# ----------------------------------------------------------------- host prep
def _pack_core(src_c, dst_c, base, cfg):
    nshard, tile_e, segcap = cfg["nshard"], cfg["tile_e"], cfg["segcap"]
    counts = np.bincount(dst_c - base, minlength=nshard)
    assert counts.min() >= 1 and counts.max() <= tile_e
    cum = np.concatenate([[0], np.cumsum(counts)])
    tiles = []
    i = 0
    while i < nshard:
        j = int(np.searchsorted(cum, cum[i] + tile_e, side="right")) - 1
        j = min(j, i + segcap, nshard)
        assert j > i
        tiles.append((i, j))
        i = j
    T = len(tiles)
    src_g = np.zeros((T, tile_e), np.int32)
    segid = np.full((T, tile_e), DUMMY_SEG, np.float32)
    dstseg = np.full((T, segcap), -1, np.int64)
    for t, (i, j) in enumerate(tiles):
        e0, e1 = int(cum[i]), int(cum[j])
        nE = e1 - e0
        src_g[t, :nE] = src_c[e0:e1]
        segid[t, :nE] = (dst_c[e0:e1] - base - i).astype(np.float32)
        dstseg[t, : j - i] = np.arange(i, j)
    tstart = np.array([i for (i, j) in tiles], np.int32)
    return src_g, segid, dstseg, tstart


def _edge_layout(arr_t):  # [T, tile_e] -> [128, T*K]: (t, k*128+p) -> [p, K*t+k]
    T, tile_e = arr_t.shape
    k = tile_e // 128
    return np.ascontiguousarray(
        arr_t.reshape(T, k, 128).transpose(2, 0, 1).reshape(128, T * k)
    )


def preprocess(edge_index, cfg):
    n, nshard, sup, segcap, k = (cfg["n"], cfg["nshard"], cfg["sup"],
                                 cfg["segcap"], cfg["k"])
    src = np.asarray(edge_index[0]).astype(np.int64)
    dst = np.asarray(edge_index[1]).astype(np.int64)
    loop = np.arange(n, dtype=np.int64)
    src = np.concatenate([src, loop])
    dst = np.concatenate([dst, loop])
    order = np.argsort(dst, kind="stable")
    src, dst = src[order], dst[order]
    bounds = np.searchsorted(dst, np.arange(N_CORES + 1) * nshard)
    packed = [
        _pack_core(src[bounds[c]: bounds[c + 1]], dst[bounds[c]: bounds[c + 1]],
                   c * nshard, cfg)
        for c in range(N_CORES)
    ]
    Tmax = max(p[0].shape[0] for p in packed)
    Tmax = -(-Tmax // sup) * sup
    cores = []
    for c, (src_g, segid, dstseg, tstart) in enumerate(packed):
        pad = Tmax - src_g.shape[0]
        src_g = np.pad(src_g, ((0, pad), (0, 0)))
        segid = np.pad(segid, ((0, pad), (0, 0)), constant_values=DUMMY_SEG)
        dstseg = np.pad(dstseg, ((0, pad), (0, 0)), constant_values=-1)
        tstart = np.pad(tstart, (0, pad))
        segs = dstseg.reshape(Tmax // sup, sup * segcap)
        scat = np.where(segs >= 0, segs, SENT).astype(np.int32)
        scat = _edge_layout(scat)  # [128, (T//sup)*4]
        # per-chunk tile starts (adrow gather indices): chunk cc -> tile cc//k
        ts1 = np.repeat(tstart, k).astype(np.int32)            # adc1 offsets
        ts2 = (np.repeat(np.arange(Tmax), k) * segcap).astype(np.int32)
        NTS = -(-Tmax * k // 128)
        ts1 = np.pad(ts1, (0, NTS * 128 - len(ts1)))
        ts2 = np.pad(ts2, (0, NTS * 128 - len(ts2)))
        cores.append(dict(
            srcg=_edge_layout(src_g),
            segid=_edge_layout(segid).astype(np.float32),
            scat=scat,
            dstseg=dstseg,
            tstart1=np.ascontiguousarray(ts1.reshape(NTS, 128).T),
            tstart2=np.ascontiguousarray(ts2.reshape(NTS, 128).T),
        ))
    return cores, Tmax


def _compress_deps(nc):
    """Drop redundant sync dependencies (walrus wait-slot pressure)."""
    f = nc.m.functions[0]

    def all_insts(blk):
        for i in blk.instructions:
            yield i
        for sb in getattr(blk, "blocks", []) or []:
            yield from all_insts(sb)

    insts = [i for b in f.blocks for i in all_insts(b)]
    pos = {i.name: p for p, i in enumerate(insts)}
    by_name = {i.name: i for i in insts}

    def stream_key(p):
        tname = type(p).__name__
        if tname == "InstCollectiveCompute":
            return None
        if tname == "InstDMACopy":
            return ("dma", str(getattr(p, "queue", "")), str(p.engine))
        return ("eng", str(p.engine))

    for i in insts:
        deps = list(i.sync_dependency_names())
        if len(deps) <= 2:
            continue
        best: dict = {}
        keep = []
        for d in deps:
            p = by_name.get(d)
            if p is None:
                keep.append(d)
                continue
            kk = stream_key(p)
            if kk is None:
                keep.append(d)
                continue
            cur = best.get(kk)
            if cur is None or pos[d] > pos[cur]:
                best[kk] = d
        keep += list(best.values())
        for d in deps:
            if d not in keep:
                i.try_remove_dependency(d)


# ------------------------------------------------------------- device program
def build_program(cfg, T):
    import concourse.bass as bass
    import concourse.bacc as bacc
    import concourse.mybir as mybir
    import concourse.tile as tile
    from concourse.masks import make_identity

    f32 = mybir.dt.float32
    bf16 = mybir.dt.bfloat16
    i32 = mybir.dt.int32
    AF = mybir.ActivationFunctionType
    OP = mybir.AluOpType
    nshard, fin, h, c = cfg["nshard"], cfg["fin"], cfg["h"], cfg["c"]
    k, segcap, sup, ntw = cfg["k"], cfg["segcap"], cfg["sup"], cfg["ntw"]
    n = cfg["n"]
    nsup = T // sup
    cps = sup * k            # 64 chunks per super-tile
    st1, st2 = h + 2, c + 2  # table row strides (66 / 42)
    rw1, rw2 = h + 1, c + 1  # row widths up to & incl a_src (65 / 41)
    qw = ntw // 4
    NTS = -(-T * k // 128)   # adrow gather call count

    nc = bacc.Bacc(
        "TRN2", target_bir_lowering=False, debug=False,
        enable_asserts=False, num_devices=N_CORES,
    )

    xT = nc.dram_tensor("xT", [fin, nshard], bf16, kind="ExternalInput").ap()
    w1aug = nc.dram_tensor("w1aug", [fin, st1], bf16, kind="ExternalInput").ap()
    w2aug = nc.dram_tensor("w2aug", [h, st2], bf16, kind="ExternalInput").ap()
    b1 = nc.dram_tensor("b1", [h, 1], f32, kind="ExternalInput").ap()
    b2r = nc.dram_tensor("b2r", [1, sup * c], f32, kind="ExternalInput").ap()
    iotaI = nc.dram_tensor("iotaI", [128, segcap], bf16,
                           kind="ExternalInput").ap()
    srcg = nc.dram_tensor("srcg", [128, T * k], i32, kind="ExternalInput").ap()
    segid = nc.dram_tensor("segid", [128, T * k], f32,
                           kind="ExternalInput").ap()
    scat = nc.dram_tensor("scat", [128, nsup * k], i32,
                          kind="ExternalInput").ap()
    tstart1 = nc.dram_tensor("tstart1", [128, NTS], i32,
                             kind="ExternalInput").ap()
    tstart2 = nc.dram_tensor("tstart2", [128, NTS], i32,
                             kind="ExternalInput").ap()
    out2 = nc.dram_tensor("out2", [nsup * segcap, sup * c], f32,
                          kind="ExternalOutput").ap()

    with tile.TileContext(nc) as tc:
        with (
            tc.tile_pool(name="consts", bufs=1) as cpool,
            tc.tile_pool(name="work", bufs=2) as wpool,
            tc.tile_pool(name="epil", bufs=3) as epool,
            tc.tile_pool(name="psum", bufs=2, space="PSUM") as pp,
            tc.tile_pool(name="dram", bufs=1, space="DRAM") as dpool,
        ):
            # ---- constants
            w1aug_sb = cpool.tile([fin, st1], bf16, name="w1aug_sb")
            nc.sync.dma_start(w1aug_sb[:], w1aug)
            w2aug_sb = cpool.tile([h, st2], bf16, name="w2aug_sb")
            nc.sync.dma_start(w2aug_sb[:], w2aug)
            b1_sb = cpool.tile([h, 1], f32, name="b1_sb")
            nc.sync.dma_start(b1_sb[:], b1)
            b2r_sb = cpool.tile([1, sup * c], f32, name="b2r_sb")
            nc.sync.dma_start(b2r_sb[:], b2r)
            b2b = cpool.tile([segcap, sup * c], f32, name="b2b")
            nc.gpsimd.partition_broadcast(b2b[:], b2r_sb[:])
            identB = cpool.tile([128, 128], bf16, name="identB")
            make_identity(nc, identB[:])
            iota_bf = cpool.tile([128, segcap], bf16, name="iota_bf")
            nc.sync.dma_start(iota_bf[:], iotaI)
            srcg_sb = cpool.tile([128, T * k], i32, name="srcg_sb")
            nc.sync.dma_start(srcg_sb[:], srcg)
            segid_sb = cpool.tile([128, T * k], f32, name="segid_sb")
            nc.sync.dma_start(segid_sb[:], segid)
            scat_sb = cpool.tile([128, nsup * k], i32, name="scat_sb")
            nc.sync.dma_start(scat_sb[:], scat)
            ts1_sb = cpool.tile([128, NTS], i32, name="ts1_sb")
            nc.sync.dma_start(ts1_sb[:], tstart1)
            ts2_sb = cpool.tile([128, NTS], i32, name="ts2_sb")
            nc.sync.dma_start(ts2_sb[:], tstart2)

            bound_reg = nc.gpsimd.to_reg(nshard - 1)

            # ---- internal DRAM tables (bf16)
            h1s = dpool.tile([nshard, st1], bf16, name="h1s")
            h1f = dpool.tile([n, st1], bf16, name="h1f", addr_space="Shared")
            g2s = dpool.tile([nshard, st2], bf16, name="g2s")
            g2f = dpool.tile([n, st2], bf16, name="g2f", addr_space="Shared")
            adc1 = dpool.tile([nshard + segcap, 1], bf16, name="adc1")
            adp2 = dpool.tile([(T + sup) * segcap, 1], bf16, name="adp2")
            adrD = [dpool.tile([NTS * 128, segcap], bf16,
                               name=f"adrD{i}") for i in range(2)]

            # ---- phase 0: h1aug shard + compact a_dst column
            for nt in range(nshard // ntw):
                o = nt * ntw
                xt = epool.tile([fin, ntw], bf16, name="xt")
                nc.sync.dma_start(xt[:], xT[:, o:o + ntw])
                psH = pp.tile([st1, ntw], f32, name="psH", tag="pA")
                nc.tensor.matmul(psH[:], lhsT=w1aug_sb[:], rhs=xt[:],
                                 start=True, stop=True)
                h1t = epool.tile([st1, ntw], bf16, name="h1t")
                nc.vector.tensor_copy(h1t[:], psH[:])
                nc.sync.dma_start(
                    adc1[o:o + ntw, :].rearrange("(z a) b -> z (a b)", z=1),
                    h1t[st1 - 1:st1, :])
                psT = pp.tile([qw, 4 * st1], bf16, name="psT", tag="pD")
                for q in range(4):
                    nc.tensor.transpose(
                        psT[:, q * st1:(q + 1) * st1],
                        in_=h1t[:, q * qw:(q + 1) * qw],
                        identity=identB[0:st1, 0:st1],
                    )
                h1r = epool.tile([qw, 4 * st1], bf16, name="h1r")
                nc.scalar.activation(h1r[:], psT[:], AF.Copy)
                for q in range(4):
                    nc.sync.dma_start(
                        h1s[o + q * qw:o + (q + 1) * qw, :],
                        h1r[:, q * st1:(q + 1) * st1],
                    )

            nc.gpsimd.collective_compute(
                "AllGather", mybir.AluOpType.bypass,
                replica_groups=[list(range(N_CORES))],
                ins=[h1s[:]], outs=[h1f[:]],
            )

            def build_adrow(src_dram, ts_sb, dst_dram):
                """adrow chunk g*128+p holds segcap consecutive a_dst values
                starting at ts[g*128+p] of the compact column src_dram."""
                for g in range(NTS):
                    adpc = epool.tile([128, segcap], bf16, name="adpc")
                    nc.gpsimd.indirect_dma_start(
                        out=adpc[:], out_offset=None,
                        in_=src_dram[:],
                        in_offset=bass.IndirectOffsetOnAxis(
                            ap=ts_sb[:, g:g + 1], axis=0),
                        element_offset=0,
                    )
                    nc.sync.dma_start(
                        dst_dram[g * 128:(g + 1) * 128, :], adpc[:])

            build_adrow(adc1, ts1_sb, adrD[0])

            # ---- edge layer
            def edge_layer(tab, adr_dram, rw, st, last):
                lay = int(last)
                for S in range(nsup):
                    c0 = cps * S
                    rows = wpool.tile([128, cps * st], bf16,
                                      name=f"rows{lay}")
                    for cc in range(cps):
                        nc.gpsimd.indirect_dma_start(
                            out=rows[:, st * cc:st * (cc + 1)],
                            out_offset=None,
                            in_=tab[:],
                            in_offset=bass.IndirectOffsetOnAxis(
                                ap=srcg_sb[:, c0 + cc:c0 + cc + 1], axis=0),
                            element_offset=0,
                        )
                    adrow = wpool.tile([1, cps * segcap], bf16,
                                       name=f"adr{lay}")
                    nc.sync.dma_start(
                        adrow[:],
                        adr_dram[c0:c0 + cps, :].rearrange(
                            "(o a) b -> o (a b)", o=1))
                    adB = wpool.tile([128, cps * segcap], bf16,
                                     name=f"adB{lay}")
                    nc.gpsimd.partition_broadcast(adB[:], adrow[:])
                    rv = rows[:].rearrange("p (m f) -> p m f", f=st)
                    asb = wpool.tile([128, cps], bf16, name=f"as{lay}")
                    nc.vector.tensor_scalar(asb[:], rv[:, :, rw - 1],
                                            15.0, -15.0,
                                            op0=OP.min, op1=OP.max)
                    nc.vector.memset(rv[:, :, rw - 1], 1.0)
                    av = asb[:]
                    asX = bass.AP(av.tensor, av.offset,
                                  [list(av.ap[0]), list(av.ap[1]),
                                   [0, segcap]])
                    esB = wpool.tile([128, cps * segcap], bf16,
                                     name=f"esB{lay}")
                    nc.vector.tensor_tensor(out=esB[:], in0=asX, in1=adB[:],
                                            op=OP.add)
                    e2B = wpool.tile([128, cps * segcap], bf16,
                                     name=f"e2B{lay}")
                    nc.vector.tensor_scalar_mul(e2B[:], esB[:], 0.2)
                    nc.vector.tensor_tensor(out=esB[:], in0=esB[:], in1=e2B[:],
                                            op=OP.max)
                    nc.vector.tensor_scalar(esB[:], esB[:], 15.0, -15.0,
                                            op0=OP.min, op1=OP.max)
                    pB = wpool.tile([128, cps * segcap], bf16, name=f"pB{lay}")
                    nc.scalar.activation(pB[:], esB[:], AF.Exp)
                    it = iota_bf[:]
                    iotaX = bass.AP(it.tensor, it.offset,
                                    [list(it.ap[0]), [0, cps], list(it.ap[1])])
                    sg = segid_sb[:, c0:c0 + cps]
                    segX = bass.AP(sg.tensor, sg.offset,
                                   [list(sg.ap[0]), list(sg.ap[1]),
                                    [0, segcap]])
                    ohB = wpool.tile([128, cps * segcap], bf16,
                                     name=f"oh{lay}")
                    nc.vector.tensor_tensor(out=ohB[:], in0=iotaX, in1=segX,
                                            op=OP.is_equal)
                    phs = wpool.tile([128, cps * segcap], bf16,
                                     name=f"phs{lay}")
                    nc.vector.tensor_tensor(out=phs[:], in0=pB[:], in1=ohB[:],
                                            op=OP.mult)
                    if not last:
                        psA = pp.tile([rw, sup * segcap], f32, name="psA",
                                      tag="pA")
                        for i in range(sup):
                            for kk in range(k):
                                cc = k * i + kk
                                nc.tensor.matmul(
                                    psA[:, segcap * i:segcap * (i + 1)],
                                    lhsT=rows[:, st * cc:st * cc + rw],
                                    rhs=phs[:, segcap * cc:segcap * (cc + 1)],
                                    start=(kk == 0), stop=(kk == k - 1))
                        denr = epool.tile([1, sup * segcap], f32, name="denr")
                        nc.vector.reciprocal(
                            denr[:], psA[rw - 1:rw, :])
                        denb = epool.tile([1, sup * segcap], bf16, name="denb")
                        nc.scalar.activation(denb[:], denr[:], AF.Copy)
                        denbB = epool.tile([h, sup * segcap], bf16,
                                           name="denbB")
                        nc.gpsimd.partition_broadcast(denbB[:], denb[:])
                        hnb = epool.tile([h, sup * segcap], bf16, name="hnb")
                        nc.vector.tensor_tensor(out=hnb[:], in0=psA[0:h, :],
                                                in1=denbB[:], op=OP.mult)
                        h2r = epool.tile([h, sup * segcap], bf16, name="h2r")
                        nc.scalar.activation(h2r[:], hnb[:], AF.Relu,
                                             bias=b1_sb[:])
                        psC = pp.tile([st2, sup * segcap], f32, name="psC",
                                      tag="pC")
                        nc.tensor.matmul(psC[:], lhsT=w2aug_sb[:], rhs=h2r[:],
                                         start=True, stop=True)
                        fin1 = epool.tile([st2, sup * segcap], bf16,
                                          name="fin1")
                        nc.vector.tensor_copy(fin1[:], psC[:])
                        nc.sync.dma_start(
                            adp2[S * sup * segcap:(S + 1) * sup * segcap, :]
                            .rearrange("(o a) b -> o (a b)", o=1),
                            fin1[st2 - 1:st2, :])
                        psD = pp.tile([128, k * st2], bf16, name="psD",
                                      tag="pD")
                        for q in range(k):
                            nc.tensor.transpose(
                                psD[:, st2 * q:st2 * (q + 1)],
                                in_=fin1[:, 128 * q:128 * (q + 1)],
                                identity=identB[0:st2, 0:st2])
                        orows = epool.tile([128, k * st2], bf16, name="orows")
                        nc.scalar.activation(orows[:], psD[:], AF.Copy)
                        for q in range(k):
                            nc.gpsimd.indirect_dma_start(
                                out=g2s[:],
                                out_offset=bass.IndirectOffsetOnAxis(
                                    ap=scat_sb[:, k * S + q:k * S + q + 1],
                                    axis=0),
                                in_=orows[:, st2 * q:st2 * (q + 1)],
                                in_offset=None,
                                bounds_check=bound_reg, oob_is_err=False,
                            )
                    else:
                        half = sup // 2
                        psE = [pp.tile([segcap, half * rw], f32,
                                       name=f"psE{j}", tag=t2)
                               for j, t2 in enumerate(("pA", "pC"))]
                        for i in range(sup):
                            ps = psE[i // half]
                            col = (i % half) * rw
                            for kk in range(k):
                                cc = k * i + kk
                                nc.tensor.matmul(
                                    ps[:, col:col + rw],
                                    lhsT=phs[:, segcap * cc:segcap * (cc + 1)],
                                    rhs=rows[:, st * cc:st * cc + rw],
                                    start=(kk == 0), stop=(kk == k - 1))
                        r2 = epool.tile([segcap, sup], f32, name="r2")
                        for j in range(2):
                            dv = psE[j][:].rearrange(
                                "p (m f) -> p m f", f=rw)[:, :, rw - 1]
                            nc.vector.reciprocal(
                                r2[:, half * j:half * (j + 1)], dv)
                        fin2 = epool.tile([segcap, sup * (rw - 1)], f32,
                                          name="fin2")
                        for i in range(sup):
                            ps = psE[i // half]
                            col = (i % half) * rw
                            nc.vector.tensor_scalar_mul(
                                fin2[:, (rw - 1) * i:(rw - 1) * (i + 1)],
                                ps[:, col:col + rw - 1], r2[:, i:i + 1])
                        nc.vector.tensor_tensor(out=fin2[:], in0=fin2[:],
                                                in1=b2b[:], op=OP.add)
                        nc.sync.dma_start(
                            out2[S * segcap:(S + 1) * segcap, :], fin2[:])

            edge_layer(h1f, adrD[0], rw1, st1, last=False)
            build_adrow(adp2, ts2_sb, adrD[1])
            nc.gpsimd.collective_compute(
                "AllGather", mybir.AluOpType.bypass,
                replica_groups=[list(range(N_CORES))],
                ins=[g2s[:]], outs=[g2f[:]],
            )
            edge_layer(g2f, adrD[1], rw2, st2, last=True)

    _compress_deps(nc)
    nc.compile()
    return nc


# ------------------------------------------------------------------ interface
def make_inmaps(inputs, cfg):
    x = np.asarray(inputs["x"], np.float32)
    W1 = np.asarray(inputs["W1"], np.float32)
    as1 = np.asarray(inputs["att_src1"], np.float32)
    ad1 = np.asarray(inputs["att_dst1"], np.float32)
    b1v = np.asarray(inputs["b1"], np.float32)
    W2 = np.asarray(inputs["W2"], np.float32)
    as2 = np.asarray(inputs["att_src2"], np.float32)
    ad2 = np.asarray(inputs["att_dst2"], np.float32)
    b2v = np.asarray(inputs["b2"], np.float32)
    cores, T = preprocess(np.asarray(inputs["edge_index"]), cfg)
    w1aug = np.concatenate([W1, (W1 @ as1)[:, None], (W1 @ ad1)[:, None]], 1)
    w2aug = np.concatenate([W2, (W2 @ as2)[:, None], (W2 @ ad2)[:, None]], 1)
    nshard, sup, segcap = cfg["nshard"], cfg["sup"], cfg["segcap"]
    b2row = np.tile(b2v, sup)[None, :]
    iota = np.tile(np.arange(segcap, dtype=np.float32), (128, 1))
    in_maps = []
    for cidx in range(N_CORES):
        co = cores[cidx]
        xs = x[cidx * nshard:(cidx + 1) * nshard]
        in_maps.append(dict(
            xT=_bf16(xs.T),
            w1aug=_bf16(w1aug),
            w2aug=_bf16(w2aug),
            b1=np.ascontiguousarray(b1v[:, None]),
            b2r=np.ascontiguousarray(b2row),
            iotaI=_bf16(iota),
            srcg=co["srcg"],
            segid=co["segid"],
            scat=co["scat"],
            tstart1=co["tstart1"],
            tstart2=co["tstart2"],
        ))
    return in_maps, T, cores


def _assemble(cores, results, cfg):
    nshard, sup, segcap, c = (cfg["nshard"], cfg["sup"], cfg["segcap"],
                              cfg["c"])
    out = np.zeros((N_CORES * nshard, c), np.float32)
    for cidx in range(N_CORES):
        co = cores[cidx]
        dstseg = co["dstseg"]                       # [Tmax, segcap]
        Tm = dstseg.shape[0]
        stg = np.asarray(results[cidx]["out2"], np.float32).reshape(
            Tm // sup, segcap, sup, c)              # [S, p, t, c]
        S_i = np.arange(Tm) // sup
        t_i = np.arange(Tm) % sup
        for t in range(Tm):
            m = dstseg[t] >= 0
            if m.any():
                nodes = dstseg[t][m] + cidx * nshard
                out[nodes] = stg[S_i[t], m, t_i[t], :]
    return out


def kernel(**inputs):
    from concourse import bass_utils

    cfg = dict(DEF_CFG)
    in_maps, T, cores = make_inmaps(inputs, cfg)
    nc = build_program(cfg, T)
    res = bass_utils.run_bass_kernel_spmd(
        nc, in_maps, core_ids=list(range(N_CORES)))
    return _assemble(cores, res.results, cfg).astype(np.float32)
